# revision 19
# baseline (speedup 1.0000x reference)
"""ONIMemoryHub kernel for 8 Trainium2 NeuronCores (Bass/Tile).

Strategy (v2):
- Selection path (projections feeding top-k similarity + the similarity
  matmuls) runs as 3-term bf16 hi/lo splits: x@W = xh@Wh + xl@Wh + xh@Wl,
  ~2^-19 relative accuracy at 3 PE cycles/row (vs 4 for fp32).
- Values path (W_ev/W_eo/W_so/W_ro, work/gate) runs in plain bf16.
- Episodic: keys projected/normalized/weighted on the owning core, packed
  hi/lo and AllGathered; each core scans all N keys for its own queries.
  Top-k attend gathers RAW ep_store rows (replicated input) and applies
  W_ev @ W_eo after the weighted sum (linearity) - no value AllGather.
- Semantic: keys stay sharded; query projections (qs) are AllGathered
  (hi/lo packed); each core scans ALL queries against its local keys and
  takes local top-4 per query; an AllToAll returns every core's candidates
  for the queries each core owns; exact merge + softmax + gather of raw
  sem_values happens on the query owner. Per-key 1/||ks|| is applied to sim
  rows pre-top-k; per-query 1/||qs|| post-merge (order-invariant).
- Host precomputes transposes and bf16 hi/lo splits of inputs/weights.

kernel(**inputs) takes FULL inputs and returns the FULL [4096, 2048] output.
"""
import math

import numpy as np
import ml_dtypes

import concourse.bass as bass
import concourse.mybir as mybir
import concourse.tile as tile
from concourse import bacc
from concourse.bass_utils import run_bass_kernel_spmd
from concourse.masks import make_identity

AF = mybir.ActivationFunctionType
AXL = mybir.AxisListType
ALU = mybir.AluOpType

NCORES = 8
B, H, N, M, S = 4096, 2048, 4096, 16384, 64
BL, NL, ML = B // NCORES, N // NCORES, M // NCORES   # 512, 512, 2048
P = 128
HT = H // P                                          # 16
NBT = BL // P                                        # 4
EP_K = 8
SEM_K = 4
LN_EPS = 1e-5
RECENCY = 0.01

F32 = mybir.dt.float32
BF16 = mybir.dt.bfloat16
U32 = mybir.dt.uint32


def build():
    nc = bacc.Bacc("TRN2", target_bir_lowering=False, debug=False,
                   num_devices=NCORES)

    def din(name, shape, dt=F32):
        return nc.dram_tensor(name, shape, dt, kind="ExternalInput").ap()

    qtin = din("qtin", [P, 2, HT, BL], BF16)
    eptin = din("eptin", [P, 2, HT, NL], BF16)
    sktin = din("sktin", [P, 2, HT, ML], BF16)
    wq_t = din("wq_t", [HT, P, 2, HT, P], BF16)
    wek_t = din("wek_t", [HT, P, 2, HT, P], BF16)
    wsq_t = din("wsq_t", [HT, P, 2, HT, P], BF16)
    wsk_t = din("wsk_t", [HT, P, 2, HT, P], BF16)
    wev_t = din("wev_t", [4, P, HT, 512], BF16)
    weo_t = din("weo_t", [4, P, HT, 512], BF16)
    wso_t = din("wso_t", [4, P, HT, 512], BF16)
    wro_t = din("wro_t", [4, P, HT, 512], BF16)
    ep_store_b = din("ep_store_b", [N, H], BF16)
    sem_values_b = din("sem_values_b", [M, H], BF16)
    wsT_b = din("wsT_b", [P, HT, S], BF16)
    work_b = din("work_b", [S, H], BF16)
    gw1_b = din("gw1_b", [P, HT, 64], BF16)
    gw2_b = din("gw2_b", [64, 3], BF16)
    ep_imp = din("ep_imp", [N])
    ep_ts = din("ep_ts", [N])
    ep_imp_s = din("ep_imp_s", [NL])
    ep_ts_s = din("ep_ts_s", [NL])
    gate_b1 = din("gate_b1", [64])
    gate_b2 = din("gate_b2", [3])
    ln_gamma = din("ln_gamma", [H])
    ln_beta = din("ln_beta", [H])
    key_base = din("key_base", [1])

    out_s = nc.dram_tensor("out_s", [BL, H], F32, kind="ExternalOutput").ap()

    with tile.TileContext(nc) as tc:
        with (
            tc.tile_pool(name="cst", bufs=1) as cst,
            tc.tile_pool(name="rows", bufs=2) as rows,
            tc.tile_pool(name="sq", bufs=2) as sqp,
            tc.tile_pool(name="simc", bufs=2) as simcp,
            tc.tile_pool(name="tiny", bufs=2) as tiny,
            tc.tile_pool(name="gath", bufs=2) as gath,
            tc.tile_pool(name="ps_mm", bufs=3, space="PSUM") as ps_mm,
            tc.tile_pool(name="ps_tr", bufs=1, space="PSUM") as ps_tr,
            tc.tile_pool(name="ps_sml", bufs=2, space="PSUM") as ps_sml,
            tc.tile_pool(name="dram", bufs=1, space="DRAM") as dram,
        ):
            ident = cst.tile([P, P], F32)
            make_identity(nc, ident[:])
            ident_b = cst.tile([P, P], BF16)
            nc.vector.tensor_copy(ident_b[:], ident[:])
            ones_col = cst.tile([P, 1], F32)
            nc.vector.memset(ones_col[:], 1.0)

            ag_ek_in = dram.tile([2 * H + 2, NL], BF16, name="ag_ek_in")
            ag_ek_out = dram.tile([NCORES * (2 * H + 2), NL], BF16,
                                  addr_space="Shared", name="ag_ek_out")
            ag_qs_in = dram.tile([2 * H, BL], BF16, name="ag_qs_in")
            ag_qs_out = dram.tile([NCORES * 2 * H, BL], BF16,
                                  addr_space="Shared", name="ag_qs_out")
            ks_dram = dram.tile([2 * H, ML], BF16, name="ks_dram")
            cand_in = dram.tile([B, 8], F32, name="cand_in")
            cand_out = dram.tile([B, 8], F32, name="cand_out")
            bounce = dram.tile([2, BL], F32, name="bounce")

            # ---------- helpers ----------
            def load_wcol(pool, w_ap, j):
                t = pool.tile([P, 2, HT, P], BF16, tag="wcol", name="wcol",
                              bufs=2)
                nc.sync.dma_start(t[:], w_ap[j])
                return t

            def mm3(ps, stat, mov, s_sl=slice(None), m_sl=slice(None)):
                """ps = sum_hi [ Sh.T Mh + Sl.T Mh + Sh.T Ml ]."""
                for hi in range(HT):
                    sh = stat[:, 0, hi, s_sl]
                    sl = stat[:, 1, hi, s_sl]
                    mh = mov[:, 0, hi, m_sl]
                    ml = mov[:, 1, hi, m_sl]
                    nc.tensor.matmul(ps, sh, mh, start=(hi == 0), stop=False)
                    nc.tensor.matmul(ps, sl, mh, start=False, stop=False)
                    nc.tensor.matmul(ps, sh, ml, start=False,
                                     stop=(hi == HT - 1))

            def finish_inv_row(psn, width, extra_row=None):
                row = rows.tile([1, 512], F32, tag="nrow", name="nrow")
                nc.vector.tensor_copy(row[:1, :width], psn[:1, :width])
                nc.scalar.sqrt(row[:1, :width], row[:1, :width])
                nc.vector.tensor_scalar_max(row[:1, :width], row[:1, :width],
                                            1e-12)
                nc.vector.reciprocal(row[:1, :width], row[:1, :width])
                if extra_row is not None:
                    nc.vector.tensor_mul(row[:1, :width], row[:1, :width],
                                         extra_row)
                return row

            def bcast_row_dram(dram_row, width, name):
                row = rows.tile([1, width], F32, tag="crow", name="crow")
                nc.sync.dma_start(row[:1, :], dram_row)
                t = cst.tile([P, width], F32, name=name)
                nc.gpsimd.partition_broadcast(t[:, :], row[:1, :])
                return t

            # =================================================================
            # Phase W: episodic recency/importance weights
            # =================================================================
            def rec_weight(imp_ap, ts_ap, shape, tagb):
                impt = rows.tile(shape, F32, tag=tagb + "i", name="impt")
                tst = rows.tile(shape, F32, tag=tagb + "t", name="tst")
                nc.sync.dma_start(impt[:shape[0], :], imp_ap)
                nc.sync.dma_start(tst[:shape[0], :], ts_ap)
                s = tst[:shape[0], :]
                nc.scalar.activation(s, s, AF.Copy, bias=0.0, scale=-1.0)
                nc.vector.tensor_scalar_add(s, s, 1.0)
                nc.scalar.activation(s, s, AF.Abs)
                nc.scalar.activation(s, s, AF.Exp, scale=-RECENCY)
                si = impt[:shape[0], :]
                nc.vector.tensor_scalar_add(si, si, 1.0)
                nc.vector.tensor_mul(si, si, s)
                return impt

            wfull = rec_weight(ep_imp.rearrange("(p c) -> p c", p=P),
                               ep_ts.rearrange("(p c) -> p c", p=P),
                               [P, N // P], "wf")
            wpart = rows.tile([P, 1], F32, tag="wpart", name="wpart")
            nc.vector.reduce_sum(wpart[:, :], wfull[:, :], axis=AXL.X)
            pssum = ps_sml.tile([1, 512], F32, tag="nrm", name="wsps", bufs=1)
            nc.tensor.matmul(pssum[:1, :1], ones_col[:], wpart[:, :],
                             start=True, stop=True)
            wsum = rows.tile([1, 1], F32, tag="wsum", name="wsum")
            nc.vector.tensor_copy(wsum[:1, :], pssum[:1, :1])
            nc.vector.tensor_scalar_add(wsum[:1, :], wsum[:1, :], 1e-8)
            nc.vector.reciprocal(wsum[:1, :], wsum[:1, :])
            wloc = rec_weight(ep_imp_s[None, :], ep_ts_s[None, :], [1, NL],
                              "wl")
            nc.vector.tensor_scalar(wloc[:1, :], wloc[:1, :], wsum[:1, :1],
                                    None, op0=ALU.mult)

            # =================================================================
            # Phase EK: project episodic keys, split (unscaled) -> AG input;
            # the w/||k|| scale row ships with the AG as 2 bf16 rows.
            # =================================================================
            with tc.tile_pool(name="ph_ek", bufs=1) as ph_ek:
                ept = ph_ek.tile([P, 2, HT, NL], BF16, tag="ept", name="ept")
                nc.sync.dma_start(ept[:], eptin)
                psn_ek = ps_sml.tile([1, 512], F32, tag="nrm", name="psn_ek",
                                     bufs=1)
                for j in range(HT):
                    wc = load_wcol(ph_ek, wek_t, j)
                    ps = ps_mm.tile([P, 512], F32, tag="mm", name="ps_ek")
                    mm3(ps[:], wc, ept)
                    st = sqp.tile([P, 2, 512], BF16, tag="ksst", name="ekst")
                    nc.scalar.activation(st[:, 0, :], ps[:], AF.Copy)
                    nc.vector.tensor_sub(st[:, 1, :], ps[:], st[:, 0, :])
                    nc.scalar.dma_start(
                        ag_ek_in[j * P:(j + 1) * P, :], st[:, 0, :])
                    nc.scalar.dma_start(
                        ag_ek_in[H + j * P:H + (j + 1) * P, :], st[:, 1, :])
                    sq = sqp.tile([P, 512], F32, tag="sq", name="sq_ek")
                    nc.scalar.square(sq[:, :], ps[:])
                    nc.tensor.matmul(psn_ek[:1, :], ones_col[:], sq[:, :],
                                     start=(j == 0), stop=(j == HT - 1))
                inv_ek = finish_inv_row(psn_ek, NL, extra_row=wloc[:1, :])
                srow = rows.tile([2, 512], BF16, tag="srow", name="srow")
                stmp = rows.tile([1, 512], F32, tag="stmp", name="stmp")
                nc.vector.tensor_copy(srow[0:1, :], inv_ek[:1, :])
                nc.vector.tensor_copy(stmp[:1, :], srow[0:1, :])
                nc.vector.tensor_sub(stmp[:1, :], inv_ek[:1, :], stmp[:1, :])
                nc.vector.tensor_copy(srow[1:2, :], stmp[:1, :])
                nc.scalar.dma_start(ag_ek_in[2 * H:2 * H + 2, :], srow[:2, :])
            nc.gpsimd.collective_compute(
                "AllGather", ALU.bypass,
                replica_groups=[list(range(NCORES))],
                ins=[ag_ek_in.opt()], outs=[ag_ek_out.opt()])

            # =================================================================
            # Phase KS: project semantic keys, split -> DRAM; norms
            # =================================================================
            bc_ks = [cst.tile([P, 512], F32, name=f"bc_ks{kc}")
                     for kc in range(4)]
            with tc.tile_pool(name="ph_ks", bufs=1) as ph_ks:
                for mc in range(ML // 512):
                    msl = slice(mc * 512, (mc + 1) * 512)
                    skt = ph_ks.tile([P, 2, HT, 512], BF16, tag="skt",
                                     name="skt", bufs=2)
                    nc.sync.dma_start(skt[:], sktin[:, :, :, msl])
                    psn = ps_sml.tile([1, 512], F32, tag="nrm",
                                      name="psn_ks", bufs=1)
                    for j in range(HT):
                        wc = load_wcol(ph_ks, wsk_t, j)
                        ps = ps_mm.tile([P, 512], F32, tag="mm",
                                        name="ps_ks")
                        mm3(ps[:], wc, skt)
                        st = sqp.tile([P, 2, 512], BF16, tag="ksst",
                                      name="ksst")
                        nc.scalar.activation(st[:, 0, :], ps[:], AF.Copy)
                        nc.vector.tensor_sub(st[:, 1, :], ps[:], st[:, 0, :])
                        nc.gpsimd.dma_start(
                            ks_dram[j * P:(j + 1) * P, msl], st[:, 0, :])
                        nc.gpsimd.dma_start(
                            ks_dram[H + j * P:H + (j + 1) * P, msl],
                            st[:, 1, :])
                        sq = sqp.tile([P, 512], F32, tag="sq", name="sq_ks")
                        nc.scalar.square(sq[:, :], ps[:])
                        nc.tensor.matmul(psn[:1, :], ones_col[:], sq[:, :],
                                         start=(j == 0), stop=(j == HT - 1))
                    inv = finish_inv_row(psn, 512)
                    nc.gpsimd.partition_broadcast(bc_ks[mc][:, :],
                                                  inv[:1, :512])

            with tc.tile_pool(name="ph_acc", bufs=1) as ph_acc:
                with tc.tile_pool(name="ph_qhl", bufs=1) as ph_qhl:
                    # =========================================================
                    # Phase Q: project queries, split (unscaled), norms
                    # =========================================================
                    q_hl = ph_qhl.tile([P, 2, HT, BL], BF16, tag="qhl",
                                       name="q_hl")
                    with tc.tile_pool(name="ph_qt", bufs=1) as ph_qt:
                        qt = ph_qt.tile([P, 2, HT, BL], BF16, tag="qt",
                                        name="qt")
                        nc.sync.dma_start(qt[:], qtin)
                        psn_q = ps_sml.tile([1, 512], F32, tag="nrm",
                                            name="psn_q", bufs=1)
                        for j in range(HT):
                            wc = load_wcol(ph_qhl, wq_t, j)
                            ps = ps_mm.tile([P, 512], F32, tag="mm",
                                            name="ps_q")
                            mm3(ps[:], wc, qt)
                            nc.scalar.activation(q_hl[:, 0, j, :], ps[:],
                                                 AF.Copy)
                            nc.vector.tensor_sub(q_hl[:, 1, j, :], ps[:],
                                                 q_hl[:, 0, j, :])
                            sq = sqp.tile([P, 512], F32, tag="sq", name="sq_q")
                            nc.scalar.square(sq[:, :], ps[:])
                            nc.tensor.matmul(psn_q[:1, :], ones_col[:],
                                             sq[:, :], start=(j == 0),
                                             stop=(j == HT - 1))
                        inv_q = finish_inv_row(psn_q, BL)
                        nc.gpsimd.dma_start(bounce[0:1, :], inv_q[:1, :])

                    # =========================================================
                    # Phase QS: semantic query projection (unscaled)
                    # =========================================================
                    with tc.tile_pool(name="ph_qs", bufs=1) as ph_qs:
                        qs_hl = ph_qs.tile([P, 2, HT, BL], BF16, tag="qshl",
                                           name="qs_hl")
                        psn_qs = ps_sml.tile([1, 512], F32, tag="nrm",
                                             name="psn_qs", bufs=1)
                        for j in range(HT):
                            wc = load_wcol(ph_qhl, wsq_t, j)
                            ps = ps_mm.tile([P, 512], F32, tag="mm",
                                            name="ps_qs")
                            mm3(ps[:], wc, q_hl)
                            nc.scalar.activation(qs_hl[:, 0, j, :], ps[:],
                                                 AF.Copy)
                            nc.vector.tensor_sub(qs_hl[:, 1, j, :], ps[:],
                                                 qs_hl[:, 0, j, :])
                            sq = sqp.tile([P, 512], F32, tag="sq",
                                          name="sq_qs")
                            nc.scalar.square(sq[:, :], ps[:])
                            nc.tensor.matmul(psn_qs[:1, :], ones_col[:],
                                             sq[:, :], start=(j == 0),
                                             stop=(j == HT - 1))
                        inv_qs = finish_inv_row(psn_qs, BL)
                        nc.gpsimd.dma_start(bounce[1:2, :], inv_qs[:1, :])
                        nc.gpsimd.dma_start(
                            ag_qs_in[0:H, :].rearrange("(hi p) c -> p hi c",
                                                       p=P),
                            qs_hl[:, 0, :, :])
                        nc.gpsimd.dma_start(
                            ag_qs_in[H:2 * H, :].rearrange(
                                "(hi p) c -> p hi c", p=P),
                            qs_hl[:, 1, :, :])
                    nc.gpsimd.collective_compute(
                        "AllGather", ALU.bypass,
                        replica_groups=[list(range(NCORES))],
                        ins=[ag_qs_in.opt()], outs=[ag_qs_out.opt()])

                    invq_p = cst.tile([P, NBT], F32, name="invq_p")
                    invqs_p = cst.tile([P, NBT], F32, name="invqs_p")
                    nc.sync.dma_start(
                        invq_p[:, :],
                        bounce[0:1, :].rearrange("o (t p) -> (o p) t", p=P))
                    nc.sync.dma_start(
                        invqs_p[:, :],
                        bounce[1:2, :].rearrange("o (t p) -> (o p) t", p=P))

                    # --- work attention + gate precompute ---
                    wsT = cst.tile([P, HT, S], BF16, name="wsT")
                    nc.sync.dma_start(wsT[:], wsT_b)
                    gw1 = cst.tile([P, HT, 64], BF16, name="gw1")
                    nc.sync.dma_start(gw1[:], gw1_b)
                    gw2 = cst.tile([64, 3], BF16, name="gw2")
                    nc.sync.dma_start(gw2[:, :], gw2_b)
                    b1bc = bcast_row_dram(gate_b1[None, :], 64, "b1bc")
                    b2bc = bcast_row_dram(gate_b2[None, :], 3, "b2bc")
                    kb_bc = bcast_row_dram(key_base[None, :], 1, "kb_bc")

                    inv_sqrt_h = 1.0 / math.sqrt(H)
                    ewT_pre = []
                    gw_pre = []
                    for bt in range(NBT):
                        qsl = slice(bt * P, (bt + 1) * P)
                        psw = ps_sml.tile([P, S], F32, tag="sml", name="pswk", bufs=1)
                        for hi in range(HT):
                            nc.tensor.matmul(
                                psw[:, :S], q_hl[:, 0, hi, qsl], wsT[:, hi, :],
                                start=(hi == 0), stop=(hi == HT - 1))
                        wmax = tiny.tile([P, 1], F32, tag="c1", name="wmax")
                        nc.vector.reduce_max(wmax[:, :], psw[:, :S],
                                             axis=AXL.X)
                        nc.vector.tensor_scalar_mul(wmax[:, :], wmax[:, :],
                                                    -inv_sqrt_h)
                        ew = tiny.tile([P, S], F32, tag="c64", name="ew")
                        nc.scalar.activation(ew[:, :], psw[:, :S], AF.Exp,
                                             bias=wmax[:, :1],
                                             scale=inv_sqrt_h)
                        zw = tiny.tile([P, 1], F32, tag="c1", name="zw")
                        nc.vector.reduce_sum(zw[:, :], ew[:, :], axis=AXL.X)
                        nc.vector.reciprocal(zw[:, :], zw[:, :])
                        nc.vector.tensor_scalar(ew[:, :], ew[:, :],
                                                zw[:, :1], None, op0=ALU.mult)
                        pset = ps_tr.tile([S, P], F32, tag="tr", name="ewtp")
                        nc.tensor.transpose(out=pset[:S, :], in_=ew[:, :],
                                            identity=ident[:])
                        ewT = cst.tile([S, P], BF16, name=f"ewT{bt}")
                        nc.vector.tensor_copy(ewT[:, :], pset[:S, :])
                        ewT_pre.append(ewT)

                        psg = ps_sml.tile([P, 64], F32, tag="sml", name="psg", bufs=1)
                        for hi in range(HT):
                            nc.tensor.matmul(
                                psg[:, :64], q_hl[:, 0, hi, qsl],
                                gw1[:, hi, :],
                                start=(hi == 0), stop=(hi == HT - 1))
                        hid = tiny.tile([P, 64], F32, tag="c64", name="hid")
                        nc.vector.tensor_add(hid[:, :], psg[:, :64],
                                             b1bc[:, :])
                        nc.scalar.activation(hid[:, :], hid[:, :], AF.Silu)
                        psht = ps_tr.tile([64, P], F32, tag="tr", name="hidtp")
                        nc.tensor.transpose(out=psht[:64, :], in_=hid[:, :],
                                            identity=ident[:])
                        hidT = tiny.tile([64, P], BF16, tag="c128",
                                         name="hidT")
                        nc.vector.tensor_copy(hidT[:, :], psht[:64, :])
                        psg2 = ps_sml.tile([P, 3], F32, tag="sml", name="psg2", bufs=1)
                        nc.tensor.matmul(psg2[:, :3], hidT[:, :], gw2[:, :],
                                         start=True, stop=True)
                        gl = cst.tile([P, 3], F32, name=f"gl{bt}")
                        nc.vector.tensor_add(gl[:, :], psg2[:, :3], b2bc[:, :])
                        gmax = tiny.tile([P, 1], F32, tag="c1", name="gmax")
                        nc.vector.reduce_max(gmax[:, :], gl[:, :], axis=AXL.X)
                        nc.vector.tensor_scalar_mul(gmax[:, :], gmax[:, :],
                                                    -1.0)
                        nc.scalar.activation(gl[:, :], gl[:, :], AF.Exp,
                                             bias=gmax[:, :1])
                        gz = tiny.tile([P, 1], F32, tag="c1", name="gz")
                        nc.vector.reduce_sum(gz[:, :], gl[:, :], axis=AXL.X)
                        nc.vector.reciprocal(gz[:, :], gz[:, :])
                        nc.vector.tensor_scalar(gl[:, :], gl[:, :],
                                                gz[:, :1], None, op0=ALU.mult)
                        gw_pre.append(gl)

                    # =========================================================
                    # Phase SIM-E: own queries x all episodic keys
                    # =========================================================
                    cand_v_e = [cst.tile([P, 128], F32, name=f"cve{bt}")
                                for bt in range(NBT)]
                    cand_i_e = [cst.tile([P, 128], F32, name=f"cie{bt}")
                                for bt in range(NBT)]
                    with tc.tile_pool(name="ph_se", bufs=2) as ph_se:
                        for slab in range(NCORES):
                            base = slab * (2 * H + 2)
                            for khalf in range(2):
                                csl = slice(khalf * 256, (khalf + 1) * 256)
                                ekg = ph_se.tile([P, 2, HT, 256], BF16,
                                                 tag="ekg", name="ekg")
                                nc.sync.dma_start(
                                    ekg[:],
                                    ag_ek_out[base:base + 2 * H,
                                              csl].rearrange(
                                        "(s hi p) c -> p s hi c", p=P, s=2))
                                srg = rows.tile([2, 512], BF16, tag="srg",
                                                name="srg")
                                nc.sync.dma_start(
                                    srg[:2, :256],
                                    ag_ek_out[base + 2 * H:base + 2 * H + 2,
                                              csl])
                                sfull = rows.tile([1, 512], F32, tag="sfl",
                                                  name="sfull")
                                nc.vector.tensor_copy(sfull[:1, :256],
                                                      srg[0:1, :256])
                                nc.vector.tensor_tensor(
                                    out=sfull[:1, :256], in0=sfull[:1, :256],
                                    in1=srg[1:2, :256], op=ALU.add)
                                bc_e = sqp.tile([P, 512], F32, tag="sq",
                                                name="bc_e")
                                nc.gpsimd.partition_broadcast(
                                    bc_e[:, :256], sfull[:1, :256])
                                cid = 2 * slab + khalf
                                for bt in range(NBT):
                                    qsl = slice(bt * P, (bt + 1) * P)
                                    ps = ps_mm.tile([P, 512], F32, tag="mm",
                                                    name="ps_se")
                                    mm3(ps[:, :256], q_hl, ekg, s_sl=qsl)
                                    sc = simcp.tile([P, 256], F32, tag="sime",
                                                    name="sc_e")
                                    nc.vector.tensor_mul(sc[:], ps[:, :256],
                                                         bc_e[:, :256])
                                    mx = tiny.tile([P, 8], F32, tag="mx",
                                                   name="mx_e")
                                    mi = tiny.tile([P, 8], U32, tag="mi",
                                                   name="mi_e")
                                    nc.vector.max(out=mx[:], in_=sc[:])
                                    nc.vector.max_index(out=mi[:],
                                                        in_max=mx[:],
                                                        in_values=sc[:])
                                    nc.vector.tensor_copy(
                                        cand_v_e[bt][:,
                                                     cid * 8:(cid + 1) * 8],
                                        mx[:])
                                    mif = tiny.tile([P, 8], F32, tag="mif",
                                                    name="mif_e")
                                    nc.vector.tensor_copy(mif[:], mi[:])
                                    nc.vector.tensor_scalar_add(
                                        cand_i_e[bt][:,
                                                     cid * 8:(cid + 1) * 8],
                                        mif[:], float(cid * 256))

                    # --- episodic top-8 merge + gather + weighted sum ---
                    acc_e_b = [ph_acc.tile([P, H], BF16, tag=f"acce{bt}",
                                           name=f"acce{bt}")
                               for bt in range(NBT)]
                    for bt in range(NBT):
                        top8 = tiny.tile([P, 8], F32, tag="c8", name="top8")
                        nc.vector.max(out=top8[:], in_=cand_v_e[bt][:])
                        idxf = tiny.tile([P, 8], F32, tag="c8b", name="idxf")
                        eqm = simcp.tile([P, 128], F32, tag="eqm", name="eqm")
                        for kk in range(EP_K):
                            nc.vector.tensor_scalar(
                                eqm[:, :], cand_v_e[bt][:],
                                top8[:, kk:kk + 1], None, op0=ALU.is_equal)
                            nc.vector.tensor_tensor(
                                out=eqm[:, :], in0=eqm[:, :],
                                in1=cand_i_e[bt][:], op=ALU.mult)
                            nc.vector.reduce_sum(idxf[:, kk:kk + 1],
                                                 eqm[:, :], axis=AXL.X)
                        idxu = tiny.tile([P, 8], U32, tag="c8u", name="idxu")
                        nc.vector.tensor_copy(idxu[:, :], idxf[:, :])
                        sc8 = tiny.tile([P, 8], F32, tag="c8c", name="sc8")
                        nc.vector.tensor_scalar(
                            sc8[:, :], top8[:, :], invq_p[:, bt:bt + 1], None,
                            op0=ALU.mult)
                        negm = tiny.tile([P, 1], F32, tag="c1", name="negm")
                        nc.vector.tensor_scalar_mul(negm[:, :], sc8[:, 0:1],
                                                    -1.0)
                        nc.scalar.activation(sc8[:, :], sc8[:, :], AF.Exp,
                                             bias=negm[:, :1])
                        zs = tiny.tile([P, 1], F32, tag="c1", name="zs")
                        nc.vector.reduce_sum(zs[:, :], sc8[:, :], axis=AXL.X)
                        nc.vector.reciprocal(zs[:, :], zs[:, :])
                        nc.vector.tensor_scalar(zs[:, :], zs[:, :],
                                                gw_pre[bt][:, 1:2], None,
                                                op0=ALU.mult)
                        nc.vector.tensor_scalar(sc8[:, :], sc8[:, :],
                                                zs[:, :1], None, op0=ALU.mult)
                        acc = simcp.tile([P, H], F32, tag="acc", name="acc_e",
                                         bufs=2)
                        nc.vector.memset(acc[:, :], 0.0)
                        for kk in range(EP_K):
                            g = gath.tile([P, H], BF16, tag="g", name="g_e")
                            nc.gpsimd.indirect_dma_start(
                                out=g[:, :], out_offset=None, in_=ep_store_b,
                                in_offset=bass.IndirectOffsetOnAxis(
                                    ap=idxu[:, kk:kk + 1], axis=0))
                            nc.vector.scalar_tensor_tensor(
                                out=acc[:, :], in0=g[:, :],
                                scalar=sc8[:, kk:kk + 1],
                                in1=acc[:, :], op0=ALU.mult, op1=ALU.add)
                        nc.vector.tensor_copy(acc_e_b[bt][:, :], acc[:, :])

                # ==== ph_qhl closed: q_hl freed ====
                # =============================================================
                # Phase SIM-S: ALL queries x local semantic keys (kc-outer)
                # =============================================================
                with tc.tile_pool(name="ph_ss", bufs=1) as ph_ss:
                    cand_sv = ph_ss.tile([P, 32 * 32], F32, tag="csv",
                                         name="cand_sv")
                    cand_si = ph_ss.tile([P, 32 * 32], F32, tag="csi",
                                         name="cand_si")
                    for kc in range(4):
                        msl = slice(kc * 512, (kc + 1) * 512)
                        ksc = ph_ss.tile([P, 2, HT, 512], BF16, tag="ksc",
                                         name="ksc")
                        nc.sync.dma_start(
                            ksc[:],
                            ks_dram[:, msl].rearrange(
                                "(s hi p) c -> p s hi c", p=P, s=2))
                        for rq2 in range(16):
                            slabq = rq2 // 2
                            base = slabq * 2 * H
                            col0 = (rq2 % 2) * 256
                            qsg = ph_ss.tile([P, 2, HT, 256], BF16, tag="qsg",
                                             name="qsg", bufs=2)
                            nc.sync.dma_start(
                                qsg[:],
                                ag_qs_out[base:base + 2 * H,
                                          col0:col0 + 256].rearrange(
                                    "(s hi p) c -> p s hi c", p=P, s=2))
                            for rq in range(2):
                                rqt = rq2 * 2 + rq
                                qssl = slice(rq * P, (rq + 1) * P)
                                ps = ps_mm.tile([P, 512], F32, tag="mm",
                                                name="ps_ss")
                                mm3(ps[:], qsg, ksc, s_sl=qssl)
                                sc = simcp.tile([P, 512], F32, tag="scs",
                                                name="sc_s")
                                nc.vector.tensor_mul(sc[:, :], ps[:],
                                                     bc_ks[kc][:, :])
                                mx = tiny.tile([P, 8], F32, tag="mx",
                                               name="mx_s")
                                mi = tiny.tile([P, 8], U32, tag="mi",
                                               name="mi_s")
                                nc.vector.max(out=mx[:], in_=sc[:])
                                nc.vector.max_index(out=mi[:], in_max=mx[:],
                                                    in_values=sc[:])
                                wsl = slice(rqt * 32 + kc * 8,
                                            rqt * 32 + (kc + 1) * 8)
                                nc.vector.tensor_copy(cand_sv[:, wsl], mx[:])
                                mif = tiny.tile([P, 8], F32, tag="mif",
                                                name="mif_s")
                                nc.vector.tensor_copy(mif[:], mi[:])
                                nc.vector.tensor_scalar_add(
                                    cand_si[:, wsl], mif[:], float(kc * 512))

                    # local top-4 per query, global index, ship via a2a
                    for rqt in range(32):
                        wsl = slice(rqt * 32, (rqt + 1) * 32)
                        top8 = tiny.tile([P, 8], F32, tag="c8", name="top8l")
                        nc.vector.max(out=top8[:], in_=cand_sv[:, wsl])
                        idxf = tiny.tile([P, 8], F32, tag="c8b", name="idxfl")
                        eqm = simcp.tile([P, 32], F32, tag="eqs", name="eqml")
                        for kk in range(SEM_K):
                            nc.vector.tensor_scalar(
                                eqm[:, :], cand_sv[:, wsl],
                                top8[:, kk:kk + 1], None, op0=ALU.is_equal)
                            nc.vector.tensor_tensor(
                                out=eqm[:, :], in0=eqm[:, :],
                                in1=cand_si[:, wsl], op=ALU.mult)
                            nc.vector.reduce_sum(idxf[:, kk:kk + 1],
                                                 eqm[:, :], axis=AXL.X)
                        p4 = tiny.tile([P, 8], F32, tag="p4", name="p4")
                        nc.vector.tensor_copy(p4[:, 0:4], top8[:, 0:4])
                        nc.vector.tensor_scalar(
                            p4[:, 4:8], idxf[:, 0:4], kb_bc[:, 0:1], None,
                            op0=ALU.add)
                        nc.gpsimd.dma_start(cand_in[rqt * P:(rqt + 1) * P, :],
                                          p4[:, :])
                nc.gpsimd.collective_compute(
                    "AllToAll", ALU.bypass,
                    replica_groups=[list(range(NCORES))],
                    ins=[cand_in.opt()], outs=[cand_out.opt()])

                # =============================================================
                # Phase FINAL
                # =============================================================
                with tc.tile_pool(name="fin", bufs=1) as fin:
                    def transpose_b(src_b, dst):
                        for hi in range(HT):
                            pst = ps_tr.tile([P, P], BF16, tag="trb16",
                                             name="trp", bufs=2)
                            nc.tensor.transpose(
                                out=pst[:], in_=src_b[:, hi * P:(hi + 1) * P],
                                identity=ident_b[:])
                            nc.vector.tensor_copy(dst[:, hi, :], pst[:])

                    def val_stage(w_ap, accT_list, out_tiles, mode,
                                  gscale=None):
                        for jc in range(4):
                            jsl = slice(jc * 512, (jc + 1) * 512)
                            wvs = []
                            for h2 in range(2):
                                wv = fin.tile([P, 8, 512], BF16, tag="wv",
                                              name="wv", bufs=2)
                                nc.sync.dma_start(
                                    wv[:], w_ap[jc][:, h2 * 8:(h2 + 1) * 8, :])
                                wvs.append(wv)
                            for bt in range(NBT):
                                ps = ps_mm.tile([P, 512], F32, tag="mm",
                                                name="ps_v")
                                for hi in range(HT):
                                    nc.tensor.matmul(
                                        ps[:], accT_list[bt][:, hi, :],
                                        wvs[hi // 8][:, hi % 8, :],
                                        start=(hi == 0),
                                        stop=(hi == HT - 1))
                                if mode == "set":
                                    nc.vector.tensor_copy(
                                        out_tiles[bt][:, jsl], ps[:])
                                else:
                                    nc.vector.tensor_add(
                                        out_tiles[bt][:, jsl],
                                        out_tiles[bt][:, jsl], ps[:])

                    # e chain: tmp_e = acc_e @ W_ev
                    accT = [fin.tile([P, HT, P], BF16, tag="accT",
                                     name=f"accT{bt}", bufs=4)
                            for bt in range(NBT)]
                    for bt in range(NBT):
                        transpose_b(acc_e_b[bt], accT[bt])
                    tmp_e = [fin.tile([P, H], BF16, tag="t16",
                                      name=f"tmpe{bt}", bufs=4)
                             for bt in range(NBT)]
                    val_stage(wev_t, accT, tmp_e, "set")
                    accT2 = [fin.tile([P, HT, P], BF16, tag="accT",
                                      name=f"accT2{bt}", bufs=4)
                             for bt in range(NBT)]
                    for bt in range(NBT):
                        transpose_b(tmp_e[bt], accT2[bt])

                    # bl = gl0 * w_out
                    bl = [fin.tile([P, H], F32, tag="f32b", name=f"bl{bt}",
                                   bufs=4)
                          for bt in range(NBT)]
                    for jc in range(4):
                        wvw = fin.tile([S, 512], BF16, tag="wvw", name="wvw",
                                       bufs=2)
                        nc.sync.dma_start(wvw[:S, :],
                                          work_b[:, jc * 512:(jc + 1) * 512])
                        jsl = slice(jc * 512, (jc + 1) * 512)
                        for bt in range(NBT):
                            ps = ps_mm.tile([P, 512], F32, tag="mm",
                                            name="ps_w")
                            nc.tensor.matmul(ps[:], ewT_pre[bt][:, :],
                                             wvw[:S, :], start=True,
                                             stop=True)
                            nc.vector.tensor_scalar(
                                bl[bt][:, jsl], ps[:], gw_pre[bt][:, 0:1],
                                None, op0=ALU.mult)

                    # bl += tmp_e @ W_eo
                    val_stage(weo_t, accT2, bl, "add")

                    # --- semantic merge + gather (after AllToAll) ---
                    acc_s_b = [ph_acc.tile([P, H], BF16, tag=f"accs{bt}",
                                           name=f"accs{bt}")
                               for bt in range(NBT)]
                    for bt in range(NBT):
                        c32v = simcp.tile([P, 32], F32, tag="eqs",
                                          name="c32v")
                        c32i = simcp.tile([P, 32], F32, tag="eqs2",
                                          name="c32i")
                        for r in range(NCORES):
                            c8 = tiny.tile([P, 8], F32, tag="p4", name="c8in")
                            nc.sync.dma_start(
                                c8[:, :],
                                cand_out[r * BL + bt * P:
                                         r * BL + (bt + 1) * P, :])
                            nc.vector.tensor_copy(c32v[:, r * 4:(r + 1) * 4],
                                                  c8[:, 0:4])
                            nc.vector.tensor_copy(c32i[:, r * 4:(r + 1) * 4],
                                                  c8[:, 4:8])
                        top8 = tiny.tile([P, 8], F32, tag="c8", name="top8s")
                        nc.vector.max(out=top8[:], in_=c32v[:])
                        idxf = tiny.tile([P, 8], F32, tag="c8b", name="idxfs")
                        eqs = simcp.tile([P, 32], F32, tag="eqs3", name="eqs")
                        for kk in range(SEM_K):
                            nc.vector.tensor_scalar(
                                eqs[:, :], c32v[:, :], top8[:, kk:kk + 1],
                                None, op0=ALU.is_equal)
                            nc.vector.tensor_tensor(out=eqs[:, :],
                                                    in0=eqs[:, :],
                                                    in1=c32i[:, :],
                                                    op=ALU.mult)
                            nc.vector.reduce_sum(idxf[:, kk:kk + 1],
                                                 eqs[:, :], axis=AXL.X)
                        idxu = tiny.tile([P, 8], U32, tag="c8u", name="idxus")
                        nc.vector.tensor_copy(idxu[:, 0:4], idxf[:, 0:4])
                        sc4 = tiny.tile([P, 4], F32, tag="c4", name="sc4")
                        nc.vector.tensor_scalar(
                            sc4[:, :], top8[:, 0:4], invqs_p[:, bt:bt + 1],
                            None, op0=ALU.mult)
                        negm = tiny.tile([P, 1], F32, tag="c1", name="negms")
                        nc.vector.tensor_scalar_mul(negm[:, :], sc4[:, 0:1],
                                                    -1.0)
                        nc.scalar.activation(sc4[:, :], sc4[:, :], AF.Exp,
                                             bias=negm[:, :1])
                        zs = tiny.tile([P, 1], F32, tag="c1", name="zss")
                        nc.vector.reduce_sum(zs[:, :], sc4[:, :], axis=AXL.X)
                        nc.vector.reciprocal(zs[:, :], zs[:, :])
                        nc.vector.tensor_scalar(zs[:, :], zs[:, :],
                                                gw_pre[bt][:, 2:3], None,
                                                op0=ALU.mult)
                        nc.vector.tensor_scalar(sc4[:, :], sc4[:, :],
                                                zs[:, :1], None, op0=ALU.mult)
                        acc = simcp.tile([P, H], F32, tag="acc", name="acc_s",
                                         bufs=2)
                        nc.vector.memset(acc[:, :], 0.0)
                        for kk in range(SEM_K):
                            g = gath.tile([P, H], BF16, tag="g", name="g_s")
                            nc.gpsimd.indirect_dma_start(
                                out=g[:, :], out_offset=None,
                                in_=sem_values_b,
                                in_offset=bass.IndirectOffsetOnAxis(
                                    ap=idxu[:, kk:kk + 1], axis=0))
                            nc.vector.scalar_tensor_tensor(
                                out=acc[:, :], in0=g[:, :],
                                scalar=sc4[:, kk:kk + 1],
                                in1=acc[:, :], op0=ALU.mult, op1=ALU.add)
                        nc.vector.tensor_copy(acc_s_b[bt][:, :], acc[:, :])

                    # bl += acc_s @ W_so
                    accT_s = [fin.tile([P, HT, P], BF16, tag="accT",
                                       name=f"accTs{bt}", bufs=4)
                              for bt in range(NBT)]
                    for bt in range(NBT):
                        transpose_b(acc_s_b[bt], accT_s[bt])
                    val_stage(wso_t, accT_s, bl, "add")

                    # xo = bl @ W_ro; out = LN(xo)*gamma+beta
                    blb = [fin.tile([P, H], BF16, tag="t16", name=f"blb{bt}",
                                    bufs=4)
                           for bt in range(NBT)]
                    for bt in range(NBT):
                        nc.vector.tensor_copy(blb[bt][:, :], bl[bt][:, :])
                    accT_bl = [fin.tile([P, HT, P], BF16, tag="accT",
                                        name=f"accTb{bt}", bufs=4)
                               for bt in range(NBT)]
                    for bt in range(NBT):
                        transpose_b(blb[bt], accT_bl[bt])
                    xo = [fin.tile([P, H], F32, tag="f32b", name=f"xo{bt}",
                                   bufs=4)
                          for bt in range(NBT)]
                    val_stage(wro_t, accT_bl, xo, "set")

                    for bt in range(NBT):
                        x = xo[bt]
                        mu = tiny.tile([P, 1], F32, tag="c1", name="mu")
                        nc.vector.reduce_sum(mu[:, :], x[:, :], axis=AXL.X)
                        nc.vector.tensor_scalar_mul(mu[:, :], mu[:, :],
                                                    -1.0 / H)
                        nc.vector.tensor_scalar(x[:, :], x[:, :], mu[:, :1],
                                                None, op0=ALU.add)
                        sqx = simcp.tile([P, H], F32, tag="acc", name="sqx",
                                         bufs=2)
                        vs = tiny.tile([P, 1], F32, tag="c1", name="vs")
                        nc.scalar.activation(sqx[:, :], x[:, :], AF.Square,
                                             accum_out=vs[:, :1])
                        nc.vector.tensor_scalar_mul(vs[:, :], vs[:, :],
                                                    1.0 / H)
                        nc.vector.tensor_scalar_add(vs[:, :], vs[:, :],
                                                    LN_EPS)
                        nc.scalar.sqrt(vs[:, :], vs[:, :])
                        nc.vector.reciprocal(vs[:, :], vs[:, :])
                        nc.vector.tensor_scalar(x[:, :], x[:, :], vs[:, :1],
                                                None, op0=ALU.mult)
                        for jc in range(4):
                            jsl = slice(jc * 512, (jc + 1) * 512)
                            gbch = sqp.tile([P, 512], F32, tag="sq",
                                            name="gbch")
                            grow = rows.tile([1, 512], F32, tag="crow",
                                             name="grow")
                            nc.sync.dma_start(grow[:1, :],
                                              ln_gamma[None, jsl])
                            nc.gpsimd.partition_broadcast(gbch[:, :],
                                                          grow[:1, :])
                            nc.vector.tensor_mul(x[:, jsl], x[:, jsl],
                                                 gbch[:, :])
                            bbch = sqp.tile([P, 512], F32, tag="sq",
                                            name="bbch")
                            brow = rows.tile([1, 512], F32, tag="crow",
                                             name="brow")
                            nc.sync.dma_start(brow[:1, :],
                                              ln_beta[None, jsl])
                            nc.gpsimd.partition_broadcast(bbch[:, :],
                                                          brow[:1, :])
                            nc.vector.tensor_add(x[:, jsl], x[:, jsl],
                                                 bbch[:, :])
                        nc.gpsimd.dma_start(out_s[bt * P:(bt + 1) * P, :],
                                          x[:, :])

    nc.finalize()
    return nc


_NC_CACHE = None


def _bf16_split(x):
    h = x.astype(ml_dtypes.bfloat16)
    l = (x - h.astype(np.float32)).astype(ml_dtypes.bfloat16)
    return h, l


def _tile_sel_weight(w):
    """[H, H] f32 -> [j, p, 2, hi, 128] bf16 hi/lo tiled."""
    h, l = _bf16_split(w)
    out = np.empty((HT, P, 2, HT, P), dtype=ml_dtypes.bfloat16)
    hr = h.reshape(HT, P, HT, P)   # [hi, p, j, c]
    lr = l.reshape(HT, P, HT, P)
    out[:, :, 0] = hr.transpose(2, 1, 0, 3)
    out[:, :, 1] = lr.transpose(2, 1, 0, 3)
    return np.ascontiguousarray(out)


def _tile_val_weight(w):
    """[H, H] f32 -> [jc, p, hi, 512] bf16."""
    b = w.astype(ml_dtypes.bfloat16)
    r = b.reshape(HT, P, 4, 512)   # [hi, p, jc, c]
    return np.ascontiguousarray(r.transpose(2, 1, 0, 3))


def _split_T(x):
    """[R, H] f32 -> [p, 2, hi, R] bf16 (transposed hi/lo)."""
    h, l = _bf16_split(x)
    R = x.shape[0]
    out = np.empty((P, 2, HT, R), dtype=ml_dtypes.bfloat16)
    out[:, 0] = h.T.reshape(HT, P, R).transpose(1, 0, 2)
    out[:, 1] = l.T.reshape(HT, P, R).transpose(1, 0, 2)
    return np.ascontiguousarray(out)


def kernel(**inputs) -> np.ndarray:
    global _NC_CACHE
    if _NC_CACHE is None:
        _NC_CACHE = build()
    nc = _NC_CACHE

    f32 = lambda x: np.ascontiguousarray(np.asarray(x), dtype=np.float32)
    query = f32(inputs["query"])
    ep_store = f32(inputs["ep_store"])
    sem_keys = f32(inputs["sem_keys"])
    work_slots = f32(inputs["work_slots"])

    shared = {
        "wq_t": _tile_sel_weight(f32(inputs["W_query"])),
        "wek_t": _tile_sel_weight(f32(inputs["W_ek"])),
        "wsq_t": _tile_sel_weight(f32(inputs["W_sq"])),
        "wsk_t": _tile_sel_weight(f32(inputs["W_sk"])),
        "wev_t": _tile_val_weight(f32(inputs["W_ev"])),
        "weo_t": _tile_val_weight(f32(inputs["W_eo"])),
        "wso_t": _tile_val_weight(f32(inputs["W_so"])),
        "wro_t": _tile_val_weight(f32(inputs["W_ro"])),
        "ep_store_b": ep_store.astype(ml_dtypes.bfloat16),
        "sem_values_b": f32(inputs["sem_values"]).astype(ml_dtypes.bfloat16),
        "wsT_b": np.ascontiguousarray(
            work_slots.T.astype(ml_dtypes.bfloat16).reshape(HT, P, S)
            .transpose(1, 0, 2)),
        "work_b": work_slots.astype(ml_dtypes.bfloat16),
        "gw1_b": np.ascontiguousarray(
            f32(inputs["gate_W1"]).astype(ml_dtypes.bfloat16)
            .reshape(HT, P, 64).transpose(1, 0, 2)),
        "gw2_b": f32(inputs["gate_W2"]).astype(ml_dtypes.bfloat16),
        "ep_imp": f32(inputs["ep_importance"]),
        "ep_ts": f32(inputs["ep_timestamps"]),
        "gate_b1": f32(inputs["gate_b1"]),
        "gate_b2": f32(inputs["gate_b2"]),
        "ln_gamma": f32(inputs["ln_gamma"]),
        "ln_beta": f32(inputs["ln_beta"]),
    }

    in_maps = []
    for c in range(NCORES):
        m = dict(shared)
        m["qtin"] = _split_T(query[c * BL:(c + 1) * BL])
        m["eptin"] = _split_T(ep_store[c * NL:(c + 1) * NL])
        m["sktin"] = _split_T(sem_keys[c * ML:(c + 1) * ML])
        m["ep_imp_s"] = f32(inputs["ep_importance"][c * NL:(c + 1) * NL])
        m["ep_ts_s"] = f32(inputs["ep_timestamps"][c * NL:(c + 1) * NL])
        m["key_base"] = np.array([c * ML], dtype=np.float32)
        in_maps.append(m)

    res = run_bass_kernel_spmd(nc, in_maps, core_ids=list(range(NCORES)))
    return np.concatenate([res.results[c]["out_s"] for c in range(NCORES)],
                          axis=0)


# revision 20
# speedup vs baseline: 1.0119x; 1.0119x over previous
"""ONIMemoryHub kernel for 8 Trainium2 NeuronCores (Bass/Tile).

Strategy (v2):
- Selection path (projections feeding top-k similarity + the similarity
  matmuls) runs as 3-term bf16 hi/lo splits: x@W = xh@Wh + xl@Wh + xh@Wl,
  ~2^-19 relative accuracy at 3 PE cycles/row (vs 4 for fp32).
- Values path (W_ev/W_eo/W_so/W_ro, work/gate) runs in plain bf16.
- Episodic: keys projected/normalized/weighted on the owning core, packed
  hi/lo and AllGathered; each core scans all N keys for its own queries.
  Top-k attend gathers RAW ep_store rows (replicated input) and applies
  W_ev @ W_eo after the weighted sum (linearity) - no value AllGather.
- Semantic: keys stay sharded; query projections (qs) are AllGathered
  (hi/lo packed); each core scans ALL queries against its local keys and
  takes local top-4 per query; an AllToAll returns every core's candidates
  for the queries each core owns; exact merge + softmax + gather of raw
  sem_values happens on the query owner. Per-key 1/||ks|| is applied to sim
  rows pre-top-k; per-query 1/||qs|| post-merge (order-invariant).
- Host precomputes transposes and bf16 hi/lo splits of inputs/weights.

kernel(**inputs) takes FULL inputs and returns the FULL [4096, 2048] output.
"""
import math

import numpy as np
import ml_dtypes

import concourse.bass as bass
import concourse.mybir as mybir
import concourse.tile as tile
from concourse import bacc
from concourse.bass_utils import run_bass_kernel_spmd
from concourse.masks import make_identity

AF = mybir.ActivationFunctionType
AXL = mybir.AxisListType
ALU = mybir.AluOpType

NCORES = 8
B, H, N, M, S = 4096, 2048, 4096, 16384, 64
BL, NL, ML = B // NCORES, N // NCORES, M // NCORES   # 512, 512, 2048
P = 128
HT = H // P                                          # 16
NBT = BL // P                                        # 4
EP_K = 8
SEM_K = 4
LN_EPS = 1e-5
RECENCY = 0.01

F32 = mybir.dt.float32
BF16 = mybir.dt.bfloat16
U32 = mybir.dt.uint32


def build():
    nc = bacc.Bacc("TRN2", target_bir_lowering=False, debug=False,
                   num_devices=NCORES)

    def din(name, shape, dt=F32):
        return nc.dram_tensor(name, shape, dt, kind="ExternalInput").ap()

    qtin = din("qtin", [P, 2, HT, BL], BF16)
    eptin = din("eptin", [P, 2, HT, NL], BF16)
    sktin = din("sktin", [P, 2, HT, ML], BF16)
    wq_t = din("wq_t", [HT, P, 2, HT, P], BF16)
    wek_t = din("wek_t", [HT, P, 2, HT, P], BF16)
    wsq_t = din("wsq_t", [HT, P, 2, HT, P], BF16)
    wsk_t = din("wsk_t", [HT, P, 2, HT, P], BF16)
    wev_t = din("wev_t", [4, P, HT, 512], BF16)
    weo_t = din("weo_t", [4, P, HT, 512], BF16)
    wso_t = din("wso_t", [4, P, HT, 512], BF16)
    wro_t = din("wro_t", [4, P, HT, 512], BF16)
    ep_store_b = din("ep_store_b", [N, H], BF16)
    sem_values_b = din("sem_values_b", [M, H], BF16)
    wsT_b = din("wsT_b", [P, HT, S], BF16)
    work_b = din("work_b", [S, H], BF16)
    gw1_b = din("gw1_b", [P, HT, 64], BF16)
    gw2_b = din("gw2_b", [64, 3], BF16)
    ep_imp = din("ep_imp", [N])
    ep_ts = din("ep_ts", [N])
    ep_imp_s = din("ep_imp_s", [NL])
    ep_ts_s = din("ep_ts_s", [NL])
    gate_b1 = din("gate_b1", [64])
    gate_b2 = din("gate_b2", [3])
    ln_gamma = din("ln_gamma", [H])
    ln_beta = din("ln_beta", [H])
    key_base = din("key_base", [1])

    out_s = nc.dram_tensor("out_s", [BL, H], F32, kind="ExternalOutput").ap()

    with tile.TileContext(nc) as tc:
        with (
            tc.tile_pool(name="cst", bufs=1) as cst,
            tc.tile_pool(name="rows", bufs=2) as rows,
            tc.tile_pool(name="sq", bufs=2) as sqp,
            tc.tile_pool(name="simc", bufs=2) as simcp,
            tc.tile_pool(name="tiny", bufs=2) as tiny,
            tc.tile_pool(name="gath", bufs=2) as gath,
            tc.tile_pool(name="ps_mm", bufs=3, space="PSUM") as ps_mm,
            tc.tile_pool(name="ps_tr", bufs=1, space="PSUM") as ps_tr,
            tc.tile_pool(name="ps_sml", bufs=2, space="PSUM") as ps_sml,
            tc.tile_pool(name="dram", bufs=1, space="DRAM") as dram,
        ):
            ident = cst.tile([P, P], F32)
            make_identity(nc, ident[:])
            ident_b = cst.tile([P, P], BF16)
            nc.vector.tensor_copy(ident_b[:], ident[:])
            ones_col = cst.tile([P, 1], F32)
            nc.vector.memset(ones_col[:], 1.0)

            ag_ek_in = dram.tile([2 * H + 2, NL], BF16, name="ag_ek_in")
            ag_ek_out = dram.tile([NCORES * (2 * H + 2), NL], BF16,
                                  addr_space="Shared", name="ag_ek_out")
            ag_qs_in = dram.tile([2 * H, BL], BF16, name="ag_qs_in")
            ag_qs_out = dram.tile([NCORES * 2 * H, BL], BF16,
                                  addr_space="Shared", name="ag_qs_out")
            ks_dram = dram.tile([2 * H, ML], BF16, name="ks_dram")
            cand_in = dram.tile([B, 8], F32, name="cand_in")
            cand_out = dram.tile([B, 8], F32, name="cand_out")
            bounce = dram.tile([2, BL], F32, name="bounce")

            # ---------- helpers ----------
            def load_wcol(pool, w_ap, j):
                t = pool.tile([P, 2, HT, P], BF16, tag="wcol", name="wcol",
                              bufs=2)
                nc.sync.dma_start(t[:], w_ap[j])
                return t

            def mm3(ps, stat, mov, s_sl=slice(None), m_sl=slice(None)):
                """ps = sum_hi [ Sh.T Mh + Sl.T Mh + Sh.T Ml ]."""
                for hi in range(HT):
                    sh = stat[:, 0, hi, s_sl]
                    sl = stat[:, 1, hi, s_sl]
                    mh = mov[:, 0, hi, m_sl]
                    ml = mov[:, 1, hi, m_sl]
                    nc.tensor.matmul(ps, sh, mh, start=(hi == 0), stop=False)
                    nc.tensor.matmul(ps, sl, mh, start=False, stop=False)
                    nc.tensor.matmul(ps, sh, ml, start=False,
                                     stop=(hi == HT - 1))

            def finish_inv_row(psn, width, extra_row=None):
                row = rows.tile([1, 512], F32, tag="nrow", name="nrow")
                nc.vector.tensor_copy(row[:1, :width], psn[:1, :width])
                nc.scalar.sqrt(row[:1, :width], row[:1, :width])
                nc.vector.tensor_scalar_max(row[:1, :width], row[:1, :width],
                                            1e-12)
                nc.vector.reciprocal(row[:1, :width], row[:1, :width])
                if extra_row is not None:
                    nc.vector.tensor_mul(row[:1, :width], row[:1, :width],
                                         extra_row)
                return row

            def bcast_row_dram(dram_row, width, name):
                row = rows.tile([1, width], F32, tag="crow", name="crow")
                nc.sync.dma_start(row[:1, :], dram_row)
                t = cst.tile([P, width], F32, name=name)
                nc.gpsimd.partition_broadcast(t[:, :], row[:1, :])
                return t

            # =================================================================
            # Phase W: episodic recency/importance weights
            # =================================================================
            def rec_weight(imp_ap, ts_ap, shape, tagb):
                impt = rows.tile(shape, F32, tag=tagb + "i", name="impt")
                tst = rows.tile(shape, F32, tag=tagb + "t", name="tst")
                nc.sync.dma_start(impt[:shape[0], :], imp_ap)
                nc.sync.dma_start(tst[:shape[0], :], ts_ap)
                s = tst[:shape[0], :]
                nc.scalar.activation(s, s, AF.Copy, bias=0.0, scale=-1.0)
                nc.vector.tensor_scalar_add(s, s, 1.0)
                nc.scalar.activation(s, s, AF.Abs)
                nc.scalar.activation(s, s, AF.Exp, scale=-RECENCY)
                si = impt[:shape[0], :]
                nc.vector.tensor_scalar_add(si, si, 1.0)
                nc.vector.tensor_mul(si, si, s)
                return impt

            wfull = rec_weight(ep_imp.rearrange("(p c) -> p c", p=P),
                               ep_ts.rearrange("(p c) -> p c", p=P),
                               [P, N // P], "wf")
            wpart = rows.tile([P, 1], F32, tag="wpart", name="wpart")
            nc.vector.reduce_sum(wpart[:, :], wfull[:, :], axis=AXL.X)
            pssum = ps_sml.tile([1, 512], F32, tag="nrm", name="wsps", bufs=1)
            nc.tensor.matmul(pssum[:1, :1], ones_col[:], wpart[:, :],
                             start=True, stop=True)
            wsum = rows.tile([1, 1], F32, tag="wsum", name="wsum")
            nc.vector.tensor_copy(wsum[:1, :], pssum[:1, :1])
            nc.vector.tensor_scalar_add(wsum[:1, :], wsum[:1, :], 1e-8)
            nc.vector.reciprocal(wsum[:1, :], wsum[:1, :])
            wloc = rec_weight(ep_imp_s[None, :], ep_ts_s[None, :], [1, NL],
                              "wl")
            nc.vector.tensor_scalar(wloc[:1, :], wloc[:1, :], wsum[:1, :1],
                                    None, op0=ALU.mult)

            # =================================================================
            # Phase EK: project episodic keys, split (unscaled) -> AG input;
            # the w/||k|| scale row ships with the AG as 2 bf16 rows.
            # =================================================================
            with tc.tile_pool(name="ph_ek", bufs=1) as ph_ek:
                ept = ph_ek.tile([P, 2, HT, NL], BF16, tag="ept", name="ept")
                nc.sync.dma_start(ept[:], eptin)
                psn_ek = ps_sml.tile([1, 512], F32, tag="nrm", name="psn_ek",
                                     bufs=1)
                for j in range(HT):
                    wc = load_wcol(ph_ek, wek_t, j)
                    ps = ps_mm.tile([P, 512], F32, tag="mm", name="ps_ek")
                    mm3(ps[:], wc, ept)
                    st = sqp.tile([P, 2, 512], BF16, tag="ksst", name="ekst")
                    nc.scalar.activation(st[:, 0, :], ps[:], AF.Copy)
                    nc.vector.tensor_sub(st[:, 1, :], ps[:], st[:, 0, :])
                    nc.scalar.dma_start(
                        ag_ek_in[j * P:(j + 1) * P, :], st[:, 0, :])
                    nc.scalar.dma_start(
                        ag_ek_in[H + j * P:H + (j + 1) * P, :], st[:, 1, :])
                    sq = sqp.tile([P, 512], F32, tag="sq", name="sq_ek")
                    nc.scalar.square(sq[:, :], ps[:])
                    nc.tensor.matmul(psn_ek[:1, :], ones_col[:], sq[:, :],
                                     start=(j == 0), stop=(j == HT - 1))
                inv_ek = finish_inv_row(psn_ek, NL, extra_row=wloc[:1, :])
                srow = rows.tile([2, 512], BF16, tag="srow", name="srow")
                stmp = rows.tile([1, 512], F32, tag="stmp", name="stmp")
                nc.vector.tensor_copy(srow[0:1, :], inv_ek[:1, :])
                nc.vector.tensor_copy(stmp[:1, :], srow[0:1, :])
                nc.vector.tensor_sub(stmp[:1, :], inv_ek[:1, :], stmp[:1, :])
                nc.vector.tensor_copy(srow[1:2, :], stmp[:1, :])
                nc.scalar.dma_start(ag_ek_in[2 * H:2 * H + 2, :], srow[:2, :])
            nc.gpsimd.collective_compute(
                "AllGather", ALU.bypass,
                replica_groups=[list(range(NCORES))],
                ins=[ag_ek_in.opt()], outs=[ag_ek_out.opt()])

            # =================================================================
            # Phase KS: project semantic keys, split -> DRAM; norms
            # =================================================================
            bc_ks = [cst.tile([P, 512], F32, name=f"bc_ks{kc}")
                     for kc in range(4)]
            with tc.tile_pool(name="ph_ks", bufs=1) as ph_ks:
                for mc in range(ML // 512):
                    msl = slice(mc * 512, (mc + 1) * 512)
                    skt = ph_ks.tile([P, 2, HT, 512], BF16, tag="skt",
                                     name="skt", bufs=2)
                    nc.sync.dma_start(skt[:], sktin[:, :, :, msl])
                    psn = ps_sml.tile([1, 512], F32, tag="nrm",
                                      name="psn_ks", bufs=1)
                    for j in range(HT):
                        wc = load_wcol(ph_ks, wsk_t, j)
                        ps = ps_mm.tile([P, 512], F32, tag="mm",
                                        name="ps_ks")
                        mm3(ps[:], wc, skt)
                        st = sqp.tile([P, 2, 512], BF16, tag="ksst",
                                      name="ksst")
                        nc.scalar.activation(st[:, 0, :], ps[:], AF.Copy)
                        nc.vector.tensor_sub(st[:, 1, :], ps[:], st[:, 0, :])
                        nc.gpsimd.dma_start(
                            ks_dram[j * P:(j + 1) * P, msl], st[:, 0, :])
                        nc.gpsimd.dma_start(
                            ks_dram[H + j * P:H + (j + 1) * P, msl],
                            st[:, 1, :])
                        sq = sqp.tile([P, 512], F32, tag="sq", name="sq_ks")
                        nc.scalar.square(sq[:, :], ps[:])
                        nc.tensor.matmul(psn[:1, :], ones_col[:], sq[:, :],
                                         start=(j == 0), stop=(j == HT - 1))
                    inv = finish_inv_row(psn, 512)
                    nc.gpsimd.partition_broadcast(bc_ks[mc][:, :],
                                                  inv[:1, :512])

            with tc.tile_pool(name="ph_acc", bufs=1) as ph_acc:
                with tc.tile_pool(name="ph_qhl", bufs=1) as ph_qhl:
                    # =========================================================
                    # Phase Q: project queries, split (unscaled), norms
                    # =========================================================
                    q_hl = ph_qhl.tile([P, 2, HT, BL], BF16, tag="qhl",
                                       name="q_hl")
                    with tc.tile_pool(name="ph_qt", bufs=1) as ph_qt:
                        qt = ph_qt.tile([P, 2, HT, BL], BF16, tag="qt",
                                        name="qt")
                        nc.sync.dma_start(qt[:], qtin)
                        psn_q = ps_sml.tile([1, 512], F32, tag="nrm",
                                            name="psn_q", bufs=1)
                        for j in range(HT):
                            wc = load_wcol(ph_qhl, wq_t, j)
                            ps = ps_mm.tile([P, 512], F32, tag="mm",
                                            name="ps_q")
                            mm3(ps[:], wc, qt)
                            nc.scalar.activation(q_hl[:, 0, j, :], ps[:],
                                                 AF.Copy)
                            nc.vector.tensor_sub(q_hl[:, 1, j, :], ps[:],
                                                 q_hl[:, 0, j, :])
                            sq = sqp.tile([P, 512], F32, tag="sq", name="sq_q")
                            nc.scalar.square(sq[:, :], ps[:])
                            nc.tensor.matmul(psn_q[:1, :], ones_col[:],
                                             sq[:, :], start=(j == 0),
                                             stop=(j == HT - 1))
                        inv_q = finish_inv_row(psn_q, BL)
                        nc.gpsimd.dma_start(bounce[0:1, :], inv_q[:1, :])

                    # =========================================================
                    # Phase QS: semantic query projection (unscaled)
                    # =========================================================
                    with tc.tile_pool(name="ph_qs", bufs=1) as ph_qs:
                        qs_hl = ph_qs.tile([P, 2, HT, BL], BF16, tag="qshl",
                                           name="qs_hl")
                        psn_qs = ps_sml.tile([1, 512], F32, tag="nrm",
                                             name="psn_qs", bufs=1)
                        for j in range(HT):
                            wc = load_wcol(ph_qhl, wsq_t, j)
                            ps = ps_mm.tile([P, 512], F32, tag="mm",
                                            name="ps_qs")
                            mm3(ps[:], wc, q_hl)
                            nc.scalar.activation(qs_hl[:, 0, j, :], ps[:],
                                                 AF.Copy)
                            nc.vector.tensor_sub(qs_hl[:, 1, j, :], ps[:],
                                                 qs_hl[:, 0, j, :])
                            sq = sqp.tile([P, 512], F32, tag="sq",
                                          name="sq_qs")
                            nc.scalar.square(sq[:, :], ps[:])
                            nc.tensor.matmul(psn_qs[:1, :], ones_col[:],
                                             sq[:, :], start=(j == 0),
                                             stop=(j == HT - 1))
                        inv_qs = finish_inv_row(psn_qs, BL)
                        nc.gpsimd.dma_start(bounce[1:2, :], inv_qs[:1, :])
                        nc.gpsimd.dma_start(
                            ag_qs_in[0:H, :].rearrange("(hi p) c -> p hi c",
                                                       p=P),
                            qs_hl[:, 0, :, :])
                        nc.gpsimd.dma_start(
                            ag_qs_in[H:2 * H, :].rearrange(
                                "(hi p) c -> p hi c", p=P),
                            qs_hl[:, 1, :, :])
                    nc.gpsimd.collective_compute(
                        "AllGather", ALU.bypass,
                        replica_groups=[list(range(NCORES))],
                        ins=[ag_qs_in.opt()], outs=[ag_qs_out.opt()])

                    invq_p = cst.tile([P, NBT], F32, name="invq_p")
                    invqs_p = cst.tile([P, NBT], F32, name="invqs_p")
                    nc.sync.dma_start(
                        invq_p[:, :],
                        bounce[0:1, :].rearrange("o (t p) -> (o p) t", p=P))
                    nc.sync.dma_start(
                        invqs_p[:, :],
                        bounce[1:2, :].rearrange("o (t p) -> (o p) t", p=P))

                    # --- work attention + gate precompute ---
                    wsT = cst.tile([P, HT, S], BF16, name="wsT")
                    nc.sync.dma_start(wsT[:], wsT_b)
                    gw1 = cst.tile([P, HT, 64], BF16, name="gw1")
                    nc.sync.dma_start(gw1[:], gw1_b)
                    gw2 = cst.tile([64, 3], BF16, name="gw2")
                    nc.sync.dma_start(gw2[:, :], gw2_b)
                    b1bc = bcast_row_dram(gate_b1[None, :], 64, "b1bc")
                    b2bc = bcast_row_dram(gate_b2[None, :], 3, "b2bc")
                    kb_bc = bcast_row_dram(key_base[None, :], 1, "kb_bc")

                    inv_sqrt_h = 1.0 / math.sqrt(H)
                    ewT_pre = []
                    gw_pre = []
                    for bt in range(NBT):
                        qsl = slice(bt * P, (bt + 1) * P)
                        psw = ps_sml.tile([P, S], F32, tag="sml", name="pswk", bufs=1)
                        for hi in range(HT):
                            nc.tensor.matmul(
                                psw[:, :S], q_hl[:, 0, hi, qsl], wsT[:, hi, :],
                                start=(hi == 0), stop=(hi == HT - 1))
                        wmax = tiny.tile([P, 1], F32, tag="c1", name="wmax")
                        nc.vector.reduce_max(wmax[:, :], psw[:, :S],
                                             axis=AXL.X)
                        nc.vector.tensor_scalar_mul(wmax[:, :], wmax[:, :],
                                                    -inv_sqrt_h)
                        ew = tiny.tile([P, S], F32, tag="c64", name="ew")
                        nc.scalar.activation(ew[:, :], psw[:, :S], AF.Exp,
                                             bias=wmax[:, :1],
                                             scale=inv_sqrt_h)
                        zw = tiny.tile([P, 1], F32, tag="c1", name="zw")
                        nc.vector.reduce_sum(zw[:, :], ew[:, :], axis=AXL.X)
                        nc.vector.reciprocal(zw[:, :], zw[:, :])
                        nc.vector.tensor_scalar(ew[:, :], ew[:, :],
                                                zw[:, :1], None, op0=ALU.mult)
                        pset = ps_tr.tile([S, P], F32, tag="tr", name="ewtp")
                        nc.tensor.transpose(out=pset[:S, :], in_=ew[:, :],
                                            identity=ident[:])
                        ewT = cst.tile([S, P], BF16, name=f"ewT{bt}")
                        nc.vector.tensor_copy(ewT[:, :], pset[:S, :])
                        ewT_pre.append(ewT)

                        psg = ps_sml.tile([P, 64], F32, tag="sml", name="psg", bufs=1)
                        for hi in range(HT):
                            nc.tensor.matmul(
                                psg[:, :64], q_hl[:, 0, hi, qsl],
                                gw1[:, hi, :],
                                start=(hi == 0), stop=(hi == HT - 1))
                        hid = tiny.tile([P, 64], F32, tag="c64", name="hid")
                        nc.vector.tensor_add(hid[:, :], psg[:, :64],
                                             b1bc[:, :])
                        nc.scalar.activation(hid[:, :], hid[:, :], AF.Silu)
                        psht = ps_tr.tile([64, P], F32, tag="tr", name="hidtp")
                        nc.tensor.transpose(out=psht[:64, :], in_=hid[:, :],
                                            identity=ident[:])
                        hidT = tiny.tile([64, P], BF16, tag="c128",
                                         name="hidT")
                        nc.vector.tensor_copy(hidT[:, :], psht[:64, :])
                        psg2 = ps_sml.tile([P, 3], F32, tag="sml", name="psg2", bufs=1)
                        nc.tensor.matmul(psg2[:, :3], hidT[:, :], gw2[:, :],
                                         start=True, stop=True)
                        gl = cst.tile([P, 3], F32, name=f"gl{bt}")
                        nc.vector.tensor_add(gl[:, :], psg2[:, :3], b2bc[:, :])
                        gmax = tiny.tile([P, 1], F32, tag="c1", name="gmax")
                        nc.vector.reduce_max(gmax[:, :], gl[:, :], axis=AXL.X)
                        nc.vector.tensor_scalar_mul(gmax[:, :], gmax[:, :],
                                                    -1.0)
                        nc.scalar.activation(gl[:, :], gl[:, :], AF.Exp,
                                             bias=gmax[:, :1])
                        gz = tiny.tile([P, 1], F32, tag="c1", name="gz")
                        nc.vector.reduce_sum(gz[:, :], gl[:, :], axis=AXL.X)
                        nc.vector.reciprocal(gz[:, :], gz[:, :])
                        nc.vector.tensor_scalar(gl[:, :], gl[:, :],
                                                gz[:, :1], None, op0=ALU.mult)
                        gw_pre.append(gl)

                    # =========================================================
                    # Phase SIM-E: own queries x all episodic keys
                    # =========================================================
                    cand_v_e = [cst.tile([P, 128], F32, name=f"cve{bt}")
                                for bt in range(NBT)]
                    cand_i_e = [cst.tile([P, 128], F32, name=f"cie{bt}")
                                for bt in range(NBT)]
                    with tc.tile_pool(name="ph_se", bufs=2) as ph_se:
                        for slab in range(NCORES):
                            base = slab * (2 * H + 2)
                            for khalf in range(2):
                                csl = slice(khalf * 256, (khalf + 1) * 256)
                                ekg = ph_se.tile([P, 2, HT, 256], BF16,
                                                 tag="ekg", name="ekg")
                                nc.gpsimd.dma_start(
                                    ekg[:],
                                    ag_ek_out[base:base + 2 * H,
                                              csl].rearrange(
                                        "(s hi p) c -> p s hi c", p=P, s=2))
                                srg = rows.tile([2, 512], BF16, tag="srg",
                                                name="srg")
                                nc.gpsimd.dma_start(
                                    srg[:2, :256],
                                    ag_ek_out[base + 2 * H:base + 2 * H + 2,
                                              csl])
                                sfull = rows.tile([1, 512], F32, tag="sfl",
                                                  name="sfull")
                                nc.vector.tensor_copy(sfull[:1, :256],
                                                      srg[0:1, :256])
                                nc.vector.tensor_tensor(
                                    out=sfull[:1, :256], in0=sfull[:1, :256],
                                    in1=srg[1:2, :256], op=ALU.add)
                                bc_e = sqp.tile([P, 512], F32, tag="sq",
                                                name="bc_e")
                                nc.gpsimd.partition_broadcast(
                                    bc_e[:, :256], sfull[:1, :256])
                                cid = 2 * slab + khalf
                                for bt in range(NBT):
                                    qsl = slice(bt * P, (bt + 1) * P)
                                    ps = ps_mm.tile([P, 512], F32, tag="mm",
                                                    name="ps_se")
                                    mm3(ps[:, :256], q_hl, ekg, s_sl=qsl)
                                    sc = simcp.tile([P, 256], F32, tag="sime",
                                                    name="sc_e")
                                    nc.vector.tensor_mul(sc[:], ps[:, :256],
                                                         bc_e[:, :256])
                                    mx = tiny.tile([P, 8], F32, tag="mx",
                                                   name="mx_e")
                                    mi = tiny.tile([P, 8], U32, tag="mi",
                                                   name="mi_e")
                                    nc.vector.max(out=mx[:], in_=sc[:])
                                    nc.vector.max_index(out=mi[:],
                                                        in_max=mx[:],
                                                        in_values=sc[:])
                                    nc.vector.tensor_copy(
                                        cand_v_e[bt][:,
                                                     cid * 8:(cid + 1) * 8],
                                        mx[:])
                                    mif = tiny.tile([P, 8], F32, tag="mif",
                                                    name="mif_e")
                                    nc.vector.tensor_copy(mif[:], mi[:])
                                    nc.vector.tensor_scalar_add(
                                        cand_i_e[bt][:,
                                                     cid * 8:(cid + 1) * 8],
                                        mif[:], float(cid * 256))

                    # --- episodic top-8 merge + gather + weighted sum ---
                    acc_e_b = [ph_acc.tile([P, H], BF16, tag=f"acce{bt}",
                                           name=f"acce{bt}")
                               for bt in range(NBT)]
                    for bt in range(NBT):
                        top8 = tiny.tile([P, 8], F32, tag="c8", name="top8")
                        nc.vector.max(out=top8[:], in_=cand_v_e[bt][:])
                        idxf = tiny.tile([P, 8], F32, tag="c8b", name="idxf")
                        eqm = simcp.tile([P, 128], F32, tag="eqm", name="eqm")
                        for kk in range(EP_K):
                            nc.vector.tensor_scalar(
                                eqm[:, :], cand_v_e[bt][:],
                                top8[:, kk:kk + 1], None, op0=ALU.is_equal)
                            nc.vector.tensor_tensor(
                                out=eqm[:, :], in0=eqm[:, :],
                                in1=cand_i_e[bt][:], op=ALU.mult)
                            nc.vector.reduce_sum(idxf[:, kk:kk + 1],
                                                 eqm[:, :], axis=AXL.X)
                        idxu = tiny.tile([P, 8], U32, tag="c8u", name="idxu")
                        nc.vector.tensor_copy(idxu[:, :], idxf[:, :])
                        sc8 = tiny.tile([P, 8], F32, tag="c8c", name="sc8")
                        nc.vector.tensor_scalar(
                            sc8[:, :], top8[:, :], invq_p[:, bt:bt + 1], None,
                            op0=ALU.mult)
                        negm = tiny.tile([P, 1], F32, tag="c1", name="negm")
                        nc.vector.tensor_scalar_mul(negm[:, :], sc8[:, 0:1],
                                                    -1.0)
                        nc.scalar.activation(sc8[:, :], sc8[:, :], AF.Exp,
                                             bias=negm[:, :1])
                        zs = tiny.tile([P, 1], F32, tag="c1", name="zs")
                        nc.vector.reduce_sum(zs[:, :], sc8[:, :], axis=AXL.X)
                        nc.vector.reciprocal(zs[:, :], zs[:, :])
                        nc.vector.tensor_scalar(zs[:, :], zs[:, :],
                                                gw_pre[bt][:, 1:2], None,
                                                op0=ALU.mult)
                        nc.vector.tensor_scalar(sc8[:, :], sc8[:, :],
                                                zs[:, :1], None, op0=ALU.mult)
                        acc = simcp.tile([P, H], F32, tag="acc", name="acc_e",
                                         bufs=2)
                        nc.vector.memset(acc[:, :], 0.0)
                        for kk in range(EP_K):
                            g = gath.tile([P, H], BF16, tag="g", name="g_e")
                            nc.gpsimd.indirect_dma_start(
                                out=g[:, :], out_offset=None, in_=ep_store_b,
                                in_offset=bass.IndirectOffsetOnAxis(
                                    ap=idxu[:, kk:kk + 1], axis=0))
                            nc.vector.scalar_tensor_tensor(
                                out=acc[:, :], in0=g[:, :],
                                scalar=sc8[:, kk:kk + 1],
                                in1=acc[:, :], op0=ALU.mult, op1=ALU.add)
                        nc.vector.tensor_copy(acc_e_b[bt][:, :], acc[:, :])

                # ==== ph_qhl closed: q_hl freed ====
                # =============================================================
                # Phase SIM-S: ALL queries x local semantic keys (kc-outer)
                # =============================================================
                with tc.tile_pool(name="ph_ss", bufs=1) as ph_ss:
                    cand_sv = ph_ss.tile([P, 32 * 32], F32, tag="csv",
                                         name="cand_sv")
                    cand_si = ph_ss.tile([P, 32 * 32], F32, tag="csi",
                                         name="cand_si")
                    for kc in range(4):
                        msl = slice(kc * 512, (kc + 1) * 512)
                        ksc = ph_ss.tile([P, 2, HT, 512], BF16, tag="ksc",
                                         name="ksc")
                        nc.sync.dma_start(
                            ksc[:],
                            ks_dram[:, msl].rearrange(
                                "(s hi p) c -> p s hi c", p=P, s=2))
                        for rq2 in range(16):
                            slabq = rq2 // 2
                            base = slabq * 2 * H
                            col0 = (rq2 % 2) * 256
                            qsg = ph_ss.tile([P, 2, HT, 256], BF16, tag="qsg",
                                             name="qsg", bufs=2)
                            nc.gpsimd.dma_start(
                                qsg[:],
                                ag_qs_out[base:base + 2 * H,
                                          col0:col0 + 256].rearrange(
                                    "(s hi p) c -> p s hi c", p=P, s=2))
                            for rq in range(2):
                                rqt = rq2 * 2 + rq
                                qssl = slice(rq * P, (rq + 1) * P)
                                ps = ps_mm.tile([P, 512], F32, tag="mm",
                                                name="ps_ss")
                                mm3(ps[:], qsg, ksc, s_sl=qssl)
                                sc = simcp.tile([P, 512], F32, tag="scs",
                                                name="sc_s")
                                nc.vector.tensor_mul(sc[:, :], ps[:],
                                                     bc_ks[kc][:, :])
                                mx = tiny.tile([P, 8], F32, tag="mx",
                                               name="mx_s")
                                mi = tiny.tile([P, 8], U32, tag="mi",
                                               name="mi_s")
                                nc.vector.max(out=mx[:], in_=sc[:])
                                nc.vector.max_index(out=mi[:], in_max=mx[:],
                                                    in_values=sc[:])
                                wsl = slice(rqt * 32 + kc * 8,
                                            rqt * 32 + (kc + 1) * 8)
                                nc.vector.tensor_copy(cand_sv[:, wsl], mx[:])
                                mif = tiny.tile([P, 8], F32, tag="mif",
                                                name="mif_s")
                                nc.vector.tensor_copy(mif[:], mi[:])
                                nc.vector.tensor_scalar_add(
                                    cand_si[:, wsl], mif[:], float(kc * 512))

                    # local top-4 per query, global index, ship via a2a
                    for rqt in range(32):
                        wsl = slice(rqt * 32, (rqt + 1) * 32)
                        top8 = tiny.tile([P, 8], F32, tag="c8", name="top8l")
                        nc.vector.max(out=top8[:], in_=cand_sv[:, wsl])
                        idxf = tiny.tile([P, 8], F32, tag="c8b", name="idxfl")
                        eqm = simcp.tile([P, 32], F32, tag="eqs", name="eqml")
                        for kk in range(SEM_K):
                            nc.vector.tensor_scalar(
                                eqm[:, :], cand_sv[:, wsl],
                                top8[:, kk:kk + 1], None, op0=ALU.is_equal)
                            nc.vector.tensor_tensor(
                                out=eqm[:, :], in0=eqm[:, :],
                                in1=cand_si[:, wsl], op=ALU.mult)
                            nc.vector.reduce_sum(idxf[:, kk:kk + 1],
                                                 eqm[:, :], axis=AXL.X)
                        p4 = tiny.tile([P, 8], F32, tag="p4", name="p4")
                        nc.vector.tensor_copy(p4[:, 0:4], top8[:, 0:4])
                        nc.vector.tensor_scalar(
                            p4[:, 4:8], idxf[:, 0:4], kb_bc[:, 0:1], None,
                            op0=ALU.add)
                        nc.gpsimd.dma_start(cand_in[rqt * P:(rqt + 1) * P, :],
                                          p4[:, :])
                nc.gpsimd.collective_compute(
                    "AllToAll", ALU.bypass,
                    replica_groups=[list(range(NCORES))],
                    ins=[cand_in.opt()], outs=[cand_out.opt()])

                # =============================================================
                # Phase FINAL
                # =============================================================
                with tc.tile_pool(name="fin", bufs=1) as fin:
                    def transpose_b(src_b, dst):
                        for hi in range(HT):
                            pst = ps_tr.tile([P, P], BF16, tag="trb16",
                                             name="trp", bufs=2)
                            nc.tensor.transpose(
                                out=pst[:], in_=src_b[:, hi * P:(hi + 1) * P],
                                identity=ident_b[:])
                            nc.vector.tensor_copy(dst[:, hi, :], pst[:])

                    def val_stage(w_ap, accT_list, out_tiles, mode,
                                  gscale=None):
                        for jc in range(4):
                            jsl = slice(jc * 512, (jc + 1) * 512)
                            wvs = []
                            for h2 in range(2):
                                wv = fin.tile([P, 8, 512], BF16, tag="wv",
                                              name="wv", bufs=2)
                                nc.sync.dma_start(
                                    wv[:], w_ap[jc][:, h2 * 8:(h2 + 1) * 8, :])
                                wvs.append(wv)
                            for bt in range(NBT):
                                ps = ps_mm.tile([P, 512], F32, tag="mm",
                                                name="ps_v")
                                for hi in range(HT):
                                    nc.tensor.matmul(
                                        ps[:], accT_list[bt][:, hi, :],
                                        wvs[hi // 8][:, hi % 8, :],
                                        start=(hi == 0),
                                        stop=(hi == HT - 1))
                                if mode == "set":
                                    nc.vector.tensor_copy(
                                        out_tiles[bt][:, jsl], ps[:])
                                else:
                                    nc.vector.tensor_add(
                                        out_tiles[bt][:, jsl],
                                        out_tiles[bt][:, jsl], ps[:])

                    # e chain: tmp_e = acc_e @ W_ev
                    accT = [fin.tile([P, HT, P], BF16, tag="accT",
                                     name=f"accT{bt}", bufs=4)
                            for bt in range(NBT)]
                    for bt in range(NBT):
                        transpose_b(acc_e_b[bt], accT[bt])
                    tmp_e = [fin.tile([P, H], BF16, tag="t16",
                                      name=f"tmpe{bt}", bufs=4)
                             for bt in range(NBT)]
                    val_stage(wev_t, accT, tmp_e, "set")
                    accT2 = [fin.tile([P, HT, P], BF16, tag="accT",
                                      name=f"accT2{bt}", bufs=4)
                             for bt in range(NBT)]
                    for bt in range(NBT):
                        transpose_b(tmp_e[bt], accT2[bt])

                    # bl = gl0 * w_out
                    bl = [fin.tile([P, H], F32, tag="f32b", name=f"bl{bt}",
                                   bufs=4)
                          for bt in range(NBT)]
                    for jc in range(4):
                        wvw = fin.tile([S, 512], BF16, tag="wvw", name="wvw",
                                       bufs=2)
                        nc.sync.dma_start(wvw[:S, :],
                                          work_b[:, jc * 512:(jc + 1) * 512])
                        jsl = slice(jc * 512, (jc + 1) * 512)
                        for bt in range(NBT):
                            ps = ps_mm.tile([P, 512], F32, tag="mm",
                                            name="ps_w")
                            nc.tensor.matmul(ps[:], ewT_pre[bt][:, :],
                                             wvw[:S, :], start=True,
                                             stop=True)
                            nc.vector.tensor_scalar(
                                bl[bt][:, jsl], ps[:], gw_pre[bt][:, 0:1],
                                None, op0=ALU.mult)

                    # bl += tmp_e @ W_eo
                    val_stage(weo_t, accT2, bl, "add")

                    # --- semantic merge + gather (after AllToAll) ---
                    acc_s_b = [ph_acc.tile([P, H], BF16, tag=f"accs{bt}",
                                           name=f"accs{bt}")
                               for bt in range(NBT)]
                    for bt in range(NBT):
                        c32v = simcp.tile([P, 32], F32, tag="eqs",
                                          name="c32v")
                        c32i = simcp.tile([P, 32], F32, tag="eqs2",
                                          name="c32i")
                        for r in range(NCORES):
                            c8 = tiny.tile([P, 8], F32, tag="p4", name="c8in")
                            nc.gpsimd.dma_start(
                                c8[:, :],
                                cand_out[r * BL + bt * P:
                                         r * BL + (bt + 1) * P, :])
                            nc.vector.tensor_copy(c32v[:, r * 4:(r + 1) * 4],
                                                  c8[:, 0:4])
                            nc.vector.tensor_copy(c32i[:, r * 4:(r + 1) * 4],
                                                  c8[:, 4:8])
                        top8 = tiny.tile([P, 8], F32, tag="c8", name="top8s")
                        nc.vector.max(out=top8[:], in_=c32v[:])
                        idxf = tiny.tile([P, 8], F32, tag="c8b", name="idxfs")
                        eqs = simcp.tile([P, 32], F32, tag="eqs3", name="eqs")
                        for kk in range(SEM_K):
                            nc.vector.tensor_scalar(
                                eqs[:, :], c32v[:, :], top8[:, kk:kk + 1],
                                None, op0=ALU.is_equal)
                            nc.vector.tensor_tensor(out=eqs[:, :],
                                                    in0=eqs[:, :],
                                                    in1=c32i[:, :],
                                                    op=ALU.mult)
                            nc.vector.reduce_sum(idxf[:, kk:kk + 1],
                                                 eqs[:, :], axis=AXL.X)
                        idxu = tiny.tile([P, 8], U32, tag="c8u", name="idxus")
                        nc.vector.tensor_copy(idxu[:, 0:4], idxf[:, 0:4])
                        sc4 = tiny.tile([P, 4], F32, tag="c4", name="sc4")
                        nc.vector.tensor_scalar(
                            sc4[:, :], top8[:, 0:4], invqs_p[:, bt:bt + 1],
                            None, op0=ALU.mult)
                        negm = tiny.tile([P, 1], F32, tag="c1", name="negms")
                        nc.vector.tensor_scalar_mul(negm[:, :], sc4[:, 0:1],
                                                    -1.0)
                        nc.scalar.activation(sc4[:, :], sc4[:, :], AF.Exp,
                                             bias=negm[:, :1])
                        zs = tiny.tile([P, 1], F32, tag="c1", name="zss")
                        nc.vector.reduce_sum(zs[:, :], sc4[:, :], axis=AXL.X)
                        nc.vector.reciprocal(zs[:, :], zs[:, :])
                        nc.vector.tensor_scalar(zs[:, :], zs[:, :],
                                                gw_pre[bt][:, 2:3], None,
                                                op0=ALU.mult)
                        nc.vector.tensor_scalar(sc4[:, :], sc4[:, :],
                                                zs[:, :1], None, op0=ALU.mult)
                        acc = simcp.tile([P, H], F32, tag="acc", name="acc_s",
                                         bufs=2)
                        nc.vector.memset(acc[:, :], 0.0)
                        for kk in range(SEM_K):
                            g = gath.tile([P, H], BF16, tag="g", name="g_s")
                            nc.gpsimd.indirect_dma_start(
                                out=g[:, :], out_offset=None,
                                in_=sem_values_b,
                                in_offset=bass.IndirectOffsetOnAxis(
                                    ap=idxu[:, kk:kk + 1], axis=0))
                            nc.vector.scalar_tensor_tensor(
                                out=acc[:, :], in0=g[:, :],
                                scalar=sc4[:, kk:kk + 1],
                                in1=acc[:, :], op0=ALU.mult, op1=ALU.add)
                        nc.vector.tensor_copy(acc_s_b[bt][:, :], acc[:, :])

                    # bl += acc_s @ W_so
                    accT_s = [fin.tile([P, HT, P], BF16, tag="accT",
                                       name=f"accTs{bt}", bufs=4)
                              for bt in range(NBT)]
                    for bt in range(NBT):
                        transpose_b(acc_s_b[bt], accT_s[bt])
                    val_stage(wso_t, accT_s, bl, "add")

                    # xo = bl @ W_ro; out = LN(xo)*gamma+beta
                    blb = [fin.tile([P, H], BF16, tag="t16", name=f"blb{bt}",
                                    bufs=4)
                           for bt in range(NBT)]
                    for bt in range(NBT):
                        nc.vector.tensor_copy(blb[bt][:, :], bl[bt][:, :])
                    accT_bl = [fin.tile([P, HT, P], BF16, tag="accT",
                                        name=f"accTb{bt}", bufs=4)
                               for bt in range(NBT)]
                    for bt in range(NBT):
                        transpose_b(blb[bt], accT_bl[bt])
                    xo = [fin.tile([P, H], F32, tag="f32b", name=f"xo{bt}",
                                   bufs=4)
                          for bt in range(NBT)]
                    val_stage(wro_t, accT_bl, xo, "set")

                    for bt in range(NBT):
                        x = xo[bt]
                        mu = tiny.tile([P, 1], F32, tag="c1", name="mu")
                        nc.vector.reduce_sum(mu[:, :], x[:, :], axis=AXL.X)
                        nc.vector.tensor_scalar_mul(mu[:, :], mu[:, :],
                                                    -1.0 / H)
                        nc.vector.tensor_scalar(x[:, :], x[:, :], mu[:, :1],
                                                None, op0=ALU.add)
                        sqx = simcp.tile([P, H], F32, tag="acc", name="sqx",
                                         bufs=2)
                        vs = tiny.tile([P, 1], F32, tag="c1", name="vs")
                        nc.scalar.activation(sqx[:, :], x[:, :], AF.Square,
                                             accum_out=vs[:, :1])
                        nc.vector.tensor_scalar_mul(vs[:, :], vs[:, :],
                                                    1.0 / H)
                        nc.vector.tensor_scalar_add(vs[:, :], vs[:, :],
                                                    LN_EPS)
                        nc.scalar.sqrt(vs[:, :], vs[:, :])
                        nc.vector.reciprocal(vs[:, :], vs[:, :])
                        nc.vector.tensor_scalar(x[:, :], x[:, :], vs[:, :1],
                                                None, op0=ALU.mult)
                        for jc in range(4):
                            jsl = slice(jc * 512, (jc + 1) * 512)
                            gbch = sqp.tile([P, 512], F32, tag="sq",
                                            name="gbch")
                            grow = rows.tile([1, 512], F32, tag="crow",
                                             name="grow")
                            nc.sync.dma_start(grow[:1, :],
                                              ln_gamma[None, jsl])
                            nc.gpsimd.partition_broadcast(gbch[:, :],
                                                          grow[:1, :])
                            nc.vector.tensor_mul(x[:, jsl], x[:, jsl],
                                                 gbch[:, :])
                            bbch = sqp.tile([P, 512], F32, tag="sq",
                                            name="bbch")
                            brow = rows.tile([1, 512], F32, tag="crow",
                                             name="brow")
                            nc.sync.dma_start(brow[:1, :],
                                              ln_beta[None, jsl])
                            nc.gpsimd.partition_broadcast(bbch[:, :],
                                                          brow[:1, :])
                            nc.vector.tensor_add(x[:, jsl], x[:, jsl],
                                                 bbch[:, :])
                        nc.gpsimd.dma_start(out_s[bt * P:(bt + 1) * P, :],
                                          x[:, :])

    nc.finalize()
    return nc


_NC_CACHE = None


def _bf16_split(x):
    h = x.astype(ml_dtypes.bfloat16)
    l = (x - h.astype(np.float32)).astype(ml_dtypes.bfloat16)
    return h, l


def _tile_sel_weight(w):
    """[H, H] f32 -> [j, p, 2, hi, 128] bf16 hi/lo tiled."""
    h, l = _bf16_split(w)
    out = np.empty((HT, P, 2, HT, P), dtype=ml_dtypes.bfloat16)
    hr = h.reshape(HT, P, HT, P)   # [hi, p, j, c]
    lr = l.reshape(HT, P, HT, P)
    out[:, :, 0] = hr.transpose(2, 1, 0, 3)
    out[:, :, 1] = lr.transpose(2, 1, 0, 3)
    return np.ascontiguousarray(out)


def _tile_val_weight(w):
    """[H, H] f32 -> [jc, p, hi, 512] bf16."""
    b = w.astype(ml_dtypes.bfloat16)
    r = b.reshape(HT, P, 4, 512)   # [hi, p, jc, c]
    return np.ascontiguousarray(r.transpose(2, 1, 0, 3))


def _split_T(x):
    """[R, H] f32 -> [p, 2, hi, R] bf16 (transposed hi/lo)."""
    h, l = _bf16_split(x)
    R = x.shape[0]
    out = np.empty((P, 2, HT, R), dtype=ml_dtypes.bfloat16)
    out[:, 0] = h.T.reshape(HT, P, R).transpose(1, 0, 2)
    out[:, 1] = l.T.reshape(HT, P, R).transpose(1, 0, 2)
    return np.ascontiguousarray(out)


def kernel(**inputs) -> np.ndarray:
    global _NC_CACHE
    if _NC_CACHE is None:
        _NC_CACHE = build()
    nc = _NC_CACHE

    f32 = lambda x: np.ascontiguousarray(np.asarray(x), dtype=np.float32)
    query = f32(inputs["query"])
    ep_store = f32(inputs["ep_store"])
    sem_keys = f32(inputs["sem_keys"])
    work_slots = f32(inputs["work_slots"])

    shared = {
        "wq_t": _tile_sel_weight(f32(inputs["W_query"])),
        "wek_t": _tile_sel_weight(f32(inputs["W_ek"])),
        "wsq_t": _tile_sel_weight(f32(inputs["W_sq"])),
        "wsk_t": _tile_sel_weight(f32(inputs["W_sk"])),
        "wev_t": _tile_val_weight(f32(inputs["W_ev"])),
        "weo_t": _tile_val_weight(f32(inputs["W_eo"])),
        "wso_t": _tile_val_weight(f32(inputs["W_so"])),
        "wro_t": _tile_val_weight(f32(inputs["W_ro"])),
        "ep_store_b": ep_store.astype(ml_dtypes.bfloat16),
        "sem_values_b": f32(inputs["sem_values"]).astype(ml_dtypes.bfloat16),
        "wsT_b": np.ascontiguousarray(
            work_slots.T.astype(ml_dtypes.bfloat16).reshape(HT, P, S)
            .transpose(1, 0, 2)),
        "work_b": work_slots.astype(ml_dtypes.bfloat16),
        "gw1_b": np.ascontiguousarray(
            f32(inputs["gate_W1"]).astype(ml_dtypes.bfloat16)
            .reshape(HT, P, 64).transpose(1, 0, 2)),
        "gw2_b": f32(inputs["gate_W2"]).astype(ml_dtypes.bfloat16),
        "ep_imp": f32(inputs["ep_importance"]),
        "ep_ts": f32(inputs["ep_timestamps"]),
        "gate_b1": f32(inputs["gate_b1"]),
        "gate_b2": f32(inputs["gate_b2"]),
        "ln_gamma": f32(inputs["ln_gamma"]),
        "ln_beta": f32(inputs["ln_beta"]),
    }

    in_maps = []
    for c in range(NCORES):
        m = dict(shared)
        m["qtin"] = _split_T(query[c * BL:(c + 1) * BL])
        m["eptin"] = _split_T(ep_store[c * NL:(c + 1) * NL])
        m["sktin"] = _split_T(sem_keys[c * ML:(c + 1) * ML])
        m["ep_imp_s"] = f32(inputs["ep_importance"][c * NL:(c + 1) * NL])
        m["ep_ts_s"] = f32(inputs["ep_timestamps"][c * NL:(c + 1) * NL])
        m["key_base"] = np.array([c * ML], dtype=np.float32)
        in_maps.append(m)

    res = run_bass_kernel_spmd(nc, in_maps, core_ids=list(range(NCORES)))
    return np.concatenate([res.results[c]["out_s"] for c in range(NCORES)],
                          axis=0)


# revision 21
# speedup vs baseline: 1.1019x; 1.0890x over previous
"""ONIMemoryHub kernel for 8 Trainium2 NeuronCores (Bass/Tile).

Strategy (v2):
- Selection path (projections feeding top-k similarity + the similarity
  matmuls) runs as 3-term bf16 hi/lo splits: x@W = xh@Wh + xl@Wh + xh@Wl,
  ~2^-19 relative accuracy at 3 PE cycles/row (vs 4 for fp32).
- Values path (W_ev/W_eo/W_so/W_ro, work/gate) runs in plain bf16.
- Episodic: keys projected/normalized/weighted on the owning core, packed
  hi/lo and AllGathered; each core scans all N keys for its own queries.
  Top-k attend gathers RAW ep_store rows (replicated input) and applies
  W_ev @ W_eo after the weighted sum (linearity) - no value AllGather.
- Semantic: keys stay sharded; query projections (qs) are AllGathered
  (hi/lo packed); each core scans ALL queries against its local keys and
  takes local top-4 per query; an AllToAll returns every core's candidates
  for the queries each core owns; exact merge + softmax + gather of raw
  sem_values happens on the query owner. Per-key 1/||ks|| is applied to sim
  rows pre-top-k; per-query 1/||qs|| post-merge (order-invariant).
- Host precomputes transposes and bf16 hi/lo splits of inputs/weights.

kernel(**inputs) takes FULL inputs and returns the FULL [4096, 2048] output.
"""
import math

import numpy as np
import ml_dtypes

import concourse.bass as bass
import concourse.mybir as mybir
import concourse.tile as tile
from concourse import bacc
from concourse.bass_utils import run_bass_kernel_spmd
from concourse.masks import make_identity

AF = mybir.ActivationFunctionType
AXL = mybir.AxisListType
ALU = mybir.AluOpType

NCORES = 8
B, H, N, M, S = 4096, 2048, 4096, 16384, 64
BL, NL, ML = B // NCORES, N // NCORES, M // NCORES   # 512, 512, 2048
P = 128
HT = H // P                                          # 16
NBT = BL // P                                        # 4
EP_K = 8
SEM_K = 4
LN_EPS = 1e-5
RECENCY = 0.01

F32 = mybir.dt.float32
BF16 = mybir.dt.bfloat16
U32 = mybir.dt.uint32


def build():
    nc = bacc.Bacc("TRN2", target_bir_lowering=False, debug=False,
                   num_devices=NCORES)

    def din(name, shape, dt=F32):
        return nc.dram_tensor(name, shape, dt, kind="ExternalInput").ap()

    qtin = din("qtin", [P, 2, HT, BL], BF16)
    eptin = din("eptin", [P, 2, HT, NL], BF16)
    sktin = din("sktin", [P, 2, HT, ML], BF16)
    wq_t = din("wq_t", [HT, P, 2, HT, P], BF16)
    wek_t = din("wek_t", [HT, P, 2, HT, P], BF16)
    wsq_t = din("wsq_t", [HT, P, 2, HT, P], BF16)
    wsk_t = din("wsk_t", [HT, P, 2, HT, P], BF16)
    wev_t = din("wev_t", [4, P, HT, 512], BF16)
    weo_t = din("weo_t", [4, P, HT, 512], BF16)
    wso_t = din("wso_t", [4, P, HT, 512], BF16)
    wro_t = din("wro_t", [4, P, HT, 512], BF16)
    ep_store_b = din("ep_store_b", [N, H], BF16)
    sem_values_b = din("sem_values_b", [M, H], BF16)
    wsT_b = din("wsT_b", [P, HT, S], BF16)
    work_b = din("work_b", [S, H], BF16)
    gw1_b = din("gw1_b", [P, HT, 64], BF16)
    gw2_b = din("gw2_b", [64, 3], BF16)
    ep_imp = din("ep_imp", [N])
    ep_ts = din("ep_ts", [N])
    ep_imp_s = din("ep_imp_s", [NL])
    ep_ts_s = din("ep_ts_s", [NL])
    gate_b1 = din("gate_b1", [64])
    gate_b2 = din("gate_b2", [3])
    ln_gamma = din("ln_gamma", [H])
    ln_beta = din("ln_beta", [H])
    key_base = din("key_base", [1])

    out_s = nc.dram_tensor("out_s", [BL, H], F32, kind="ExternalOutput").ap()

    with tile.TileContext(nc) as tc:
        with (
            tc.tile_pool(name="cst", bufs=1) as cst,
            tc.tile_pool(name="rows", bufs=2) as rows,
            tc.tile_pool(name="sq", bufs=2) as sqp,
            tc.tile_pool(name="simc", bufs=2) as simcp,
            tc.tile_pool(name="tiny", bufs=2) as tiny,
            tc.tile_pool(name="gath", bufs=2) as gath,
            tc.tile_pool(name="ps_mm", bufs=3, space="PSUM") as ps_mm,
            tc.tile_pool(name="ps_tr", bufs=1, space="PSUM") as ps_tr,
            tc.tile_pool(name="ps_sml", bufs=2, space="PSUM") as ps_sml,
            tc.tile_pool(name="dram", bufs=1, space="DRAM") as dram,
        ):
            ident = cst.tile([P, P], F32)
            make_identity(nc, ident[:])
            ident_b = cst.tile([P, P], BF16)
            nc.vector.tensor_copy(ident_b[:], ident[:])
            ones_col = cst.tile([P, 1], F32)
            nc.vector.memset(ones_col[:], 1.0)

            ag_ek_in = dram.tile([2 * H + 2, NL], BF16, name="ag_ek_in")
            ag_ek_out = dram.tile([NCORES * (2 * H + 2), NL], BF16,
                                  addr_space="Shared", name="ag_ek_out")
            ag_qs_in = dram.tile([2 * H, BL], BF16, name="ag_qs_in")
            ag_qs_out = dram.tile([NCORES * 2 * H, BL], BF16,
                                  addr_space="Shared", name="ag_qs_out")
            ks_dram = dram.tile([2 * H, ML], BF16, name="ks_dram")
            cand_in = dram.tile([B, 8], F32, name="cand_in")
            cand_out = dram.tile([B, 8], F32, name="cand_out")
            bounce = dram.tile([2, BL], F32, name="bounce")

            # ---------- helpers ----------
            def load_wcol(pool, w_ap, j):
                t = pool.tile([P, 2, HT, P], BF16, tag="wcol", name="wcol",
                              bufs=2)
                nc.sync.dma_start(t[:], w_ap[j])
                return t

            def mm3(ps, stat, mov, s_sl=slice(None), m_sl=slice(None)):
                """ps = sum_hi [ Sh.T Mh + Sl.T Mh + Sh.T Ml ]."""
                for hi in range(HT):
                    sh = stat[:, 0, hi, s_sl]
                    sl = stat[:, 1, hi, s_sl]
                    mh = mov[:, 0, hi, m_sl]
                    ml = mov[:, 1, hi, m_sl]
                    nc.tensor.matmul(ps, sh, mh, start=(hi == 0), stop=False)
                    nc.tensor.matmul(ps, sl, mh, start=False, stop=False)
                    nc.tensor.matmul(ps, sh, ml, start=False,
                                     stop=(hi == HT - 1))

            def finish_inv_row(psn, width, extra_row=None):
                row = rows.tile([1, 512], F32, tag="nrow", name="nrow")
                nc.vector.tensor_copy(row[:1, :width], psn[:1, :width])
                nc.scalar.sqrt(row[:1, :width], row[:1, :width])
                nc.vector.tensor_scalar_max(row[:1, :width], row[:1, :width],
                                            1e-12)
                nc.vector.reciprocal(row[:1, :width], row[:1, :width])
                if extra_row is not None:
                    nc.vector.tensor_mul(row[:1, :width], row[:1, :width],
                                         extra_row)
                return row

            def bcast_row_dram(dram_row, width, name):
                row = rows.tile([1, width], F32, tag="crow", name="crow")
                nc.sync.dma_start(row[:1, :], dram_row)
                t = cst.tile([P, width], F32, name=name)
                nc.gpsimd.partition_broadcast(t[:, :], row[:1, :])
                return t

            # =================================================================
            # Phase W: episodic recency/importance weights
            # =================================================================
            def rec_weight(imp_ap, ts_ap, shape, tagb):
                impt = rows.tile(shape, F32, tag=tagb + "i", name="impt")
                tst = rows.tile(shape, F32, tag=tagb + "t", name="tst")
                nc.sync.dma_start(impt[:shape[0], :], imp_ap)
                nc.sync.dma_start(tst[:shape[0], :], ts_ap)
                s = tst[:shape[0], :]
                nc.scalar.activation(s, s, AF.Copy, bias=0.0, scale=-1.0)
                nc.vector.tensor_scalar_add(s, s, 1.0)
                nc.scalar.activation(s, s, AF.Abs)
                nc.scalar.activation(s, s, AF.Exp, scale=-RECENCY)
                si = impt[:shape[0], :]
                nc.vector.tensor_scalar_add(si, si, 1.0)
                nc.vector.tensor_mul(si, si, s)
                return impt

            wfull = rec_weight(ep_imp.rearrange("(p c) -> p c", p=P),
                               ep_ts.rearrange("(p c) -> p c", p=P),
                               [P, N // P], "wf")
            wpart = rows.tile([P, 1], F32, tag="wpart", name="wpart")
            nc.vector.reduce_sum(wpart[:, :], wfull[:, :], axis=AXL.X)
            pssum = ps_sml.tile([1, 512], F32, tag="nrm", name="wsps", bufs=1)
            nc.tensor.matmul(pssum[:1, :1], ones_col[:], wpart[:, :],
                             start=True, stop=True)
            wsum = rows.tile([1, 1], F32, tag="wsum", name="wsum")
            nc.vector.tensor_copy(wsum[:1, :], pssum[:1, :1])
            nc.vector.tensor_scalar_add(wsum[:1, :], wsum[:1, :], 1e-8)
            nc.vector.reciprocal(wsum[:1, :], wsum[:1, :])
            wloc = rec_weight(ep_imp_s[None, :], ep_ts_s[None, :], [1, NL],
                              "wl")
            nc.vector.tensor_scalar(wloc[:1, :], wloc[:1, :], wsum[:1, :1],
                                    None, op0=ALU.mult)

            # =================================================================
            # Phase EK: project episodic keys, split (unscaled) -> AG input;
            # the w/||k|| scale row ships with the AG as 2 bf16 rows.
            # =================================================================
            with tc.tile_pool(name="ph_ek", bufs=1) as ph_ek:
                ept = ph_ek.tile([P, 2, HT, NL], BF16, tag="ept", name="ept")
                nc.sync.dma_start(ept[:], eptin)
                psn_ek = ps_sml.tile([1, 512], F32, tag="nrm", name="psn_ek",
                                     bufs=1)
                for j in range(HT):
                    wc = load_wcol(ph_ek, wek_t, j)
                    ps = ps_mm.tile([P, 512], F32, tag="mm", name="ps_ek")
                    mm3(ps[:], wc, ept)
                    st = sqp.tile([P, 2, 512], BF16, tag="ksst", name="ekst")
                    nc.scalar.activation(st[:, 0, :], ps[:], AF.Copy)
                    nc.vector.tensor_sub(st[:, 1, :], ps[:], st[:, 0, :])
                    nc.scalar.dma_start(
                        ag_ek_in[j * P:(j + 1) * P, :], st[:, 0, :])
                    nc.scalar.dma_start(
                        ag_ek_in[H + j * P:H + (j + 1) * P, :], st[:, 1, :])
                    sq = sqp.tile([P, 512], F32, tag="sq", name="sq_ek")
                    nc.scalar.square(sq[:, :], ps[:])
                    nc.tensor.matmul(psn_ek[:1, :], ones_col[:], sq[:, :],
                                     start=(j == 0), stop=(j == HT - 1))
                inv_ek = finish_inv_row(psn_ek, NL, extra_row=wloc[:1, :])
                srow = rows.tile([2, 512], BF16, tag="srow", name="srow")
                stmp = rows.tile([1, 512], F32, tag="stmp", name="stmp")
                nc.vector.tensor_copy(srow[0:1, :], inv_ek[:1, :])
                nc.vector.tensor_copy(stmp[:1, :], srow[0:1, :])
                nc.vector.tensor_sub(stmp[:1, :], inv_ek[:1, :], stmp[:1, :])
                nc.vector.tensor_copy(srow[1:2, :], stmp[:1, :])
                nc.scalar.dma_start(ag_ek_in[2 * H:2 * H + 2, :], srow[:2, :])
            nc.gpsimd.collective_compute(
                "AllGather", ALU.bypass,
                replica_groups=[list(range(NCORES))],
                ins=[ag_ek_in.opt()], outs=[ag_ek_out.opt()])

            # =================================================================
            # Phase KS: project semantic keys, split -> DRAM; norms
            # =================================================================
            bc_ks = [cst.tile([P, 512], F32, name=f"bc_ks{kc}")
                     for kc in range(4)]
            with tc.tile_pool(name="ph_ks", bufs=1) as ph_ks:
                for mc in range(ML // 512):
                    msl = slice(mc * 512, (mc + 1) * 512)
                    skt = ph_ks.tile([P, 2, HT, 512], BF16, tag="skt",
                                     name="skt", bufs=2)
                    nc.sync.dma_start(skt[:], sktin[:, :, :, msl])
                    psn = ps_sml.tile([1, 512], F32, tag="nrm",
                                      name="psn_ks", bufs=1)
                    for j in range(HT):
                        wc = load_wcol(ph_ks, wsk_t, j)
                        ps = ps_mm.tile([P, 512], F32, tag="mm",
                                        name="ps_ks")
                        mm3(ps[:], wc, skt)
                        st = sqp.tile([P, 2, 512], BF16, tag="ksst",
                                      name="ksst")
                        nc.scalar.activation(st[:, 0, :], ps[:], AF.Copy)
                        nc.vector.tensor_sub(st[:, 1, :], ps[:], st[:, 0, :])
                        nc.gpsimd.dma_start(
                            ks_dram[j * P:(j + 1) * P, msl], st[:, 0, :])
                        nc.gpsimd.dma_start(
                            ks_dram[H + j * P:H + (j + 1) * P, msl],
                            st[:, 1, :])
                        sq = sqp.tile([P, 512], F32, tag="sq", name="sq_ks")
                        nc.scalar.square(sq[:, :], ps[:])
                        nc.tensor.matmul(psn[:1, :], ones_col[:], sq[:, :],
                                         start=(j == 0), stop=(j == HT - 1))
                    inv = finish_inv_row(psn, 512)
                    nc.gpsimd.partition_broadcast(bc_ks[mc][:, :],
                                                  inv[:1, :512])

            with tc.tile_pool(name="ph_acc", bufs=1) as ph_acc:
                with tc.tile_pool(name="ph_qhl", bufs=1) as ph_qhl:
                    # =========================================================
                    # Phase Q: project queries, split (unscaled), norms
                    # =========================================================
                    q_hl = ph_qhl.tile([P, 2, HT, BL], BF16, tag="qhl",
                                       name="q_hl")
                    with tc.tile_pool(name="ph_qt", bufs=1) as ph_qt:
                        qt = ph_qt.tile([P, 2, HT, BL], BF16, tag="qt",
                                        name="qt")
                        nc.sync.dma_start(qt[:], qtin)
                        psn_q = ps_sml.tile([1, 512], F32, tag="nrm",
                                            name="psn_q", bufs=1)
                        for j in range(HT):
                            wc = load_wcol(ph_qhl, wq_t, j)
                            ps = ps_mm.tile([P, 512], F32, tag="mm",
                                            name="ps_q")
                            mm3(ps[:], wc, qt)
                            nc.scalar.activation(q_hl[:, 0, j, :], ps[:],
                                                 AF.Copy)
                            nc.vector.tensor_sub(q_hl[:, 1, j, :], ps[:],
                                                 q_hl[:, 0, j, :])
                            sq = sqp.tile([P, 512], F32, tag="sq", name="sq_q")
                            nc.scalar.square(sq[:, :], ps[:])
                            nc.tensor.matmul(psn_q[:1, :], ones_col[:],
                                             sq[:, :], start=(j == 0),
                                             stop=(j == HT - 1))
                        inv_q = finish_inv_row(psn_q, BL)
                        nc.gpsimd.dma_start(bounce[0:1, :], inv_q[:1, :])

                    # =========================================================
                    # Phase QS: semantic query projection (unscaled)
                    # =========================================================
                    with tc.tile_pool(name="ph_qs", bufs=1) as ph_qs:
                        qs_hl = ph_qs.tile([P, 2, HT, BL], BF16, tag="qshl",
                                           name="qs_hl")
                        psn_qs = ps_sml.tile([1, 512], F32, tag="nrm",
                                             name="psn_qs", bufs=1)
                        for j in range(HT):
                            wc = load_wcol(ph_qhl, wsq_t, j)
                            ps = ps_mm.tile([P, 512], F32, tag="mm",
                                            name="ps_qs")
                            mm3(ps[:], wc, q_hl)
                            nc.scalar.activation(qs_hl[:, 0, j, :], ps[:],
                                                 AF.Copy)
                            nc.vector.tensor_sub(qs_hl[:, 1, j, :], ps[:],
                                                 qs_hl[:, 0, j, :])
                            sq = sqp.tile([P, 512], F32, tag="sq",
                                          name="sq_qs")
                            nc.scalar.square(sq[:, :], ps[:])
                            nc.tensor.matmul(psn_qs[:1, :], ones_col[:],
                                             sq[:, :], start=(j == 0),
                                             stop=(j == HT - 1))
                        inv_qs = finish_inv_row(psn_qs, BL)
                        nc.gpsimd.dma_start(bounce[1:2, :], inv_qs[:1, :])
                        nc.gpsimd.dma_start(
                            ag_qs_in[0:H, :].rearrange("(hi p) c -> p hi c",
                                                       p=P),
                            qs_hl[:, 0, :, :])
                        nc.gpsimd.dma_start(
                            ag_qs_in[H:2 * H, :].rearrange(
                                "(hi p) c -> p hi c", p=P),
                            qs_hl[:, 1, :, :])
                    nc.gpsimd.collective_compute(
                        "AllGather", ALU.bypass,
                        replica_groups=[list(range(NCORES))],
                        ins=[ag_qs_in.opt()], outs=[ag_qs_out.opt()])

                    invq_p = cst.tile([P, NBT], F32, name="invq_p")
                    invqs_p = cst.tile([P, NBT], F32, name="invqs_p")
                    nc.sync.dma_start(
                        invq_p[:, :],
                        bounce[0:1, :].rearrange("o (t p) -> (o p) t", p=P))
                    nc.sync.dma_start(
                        invqs_p[:, :],
                        bounce[1:2, :].rearrange("o (t p) -> (o p) t", p=P))

                    # --- work attention + gate precompute ---
                    wsT = cst.tile([P, HT, S], BF16, name="wsT")
                    nc.sync.dma_start(wsT[:], wsT_b)
                    gw1 = cst.tile([P, HT, 64], BF16, name="gw1")
                    nc.sync.dma_start(gw1[:], gw1_b)
                    gw2 = cst.tile([64, 3], BF16, name="gw2")
                    nc.sync.dma_start(gw2[:, :], gw2_b)
                    b1bc = bcast_row_dram(gate_b1[None, :], 64, "b1bc")
                    b2bc = bcast_row_dram(gate_b2[None, :], 3, "b2bc")
                    kb_bc = bcast_row_dram(key_base[None, :], 1, "kb_bc")

                    inv_sqrt_h = 1.0 / math.sqrt(H)
                    ewT_pre = []
                    gw_pre = []
                    for bt in range(NBT):
                        qsl = slice(bt * P, (bt + 1) * P)
                        psw = ps_sml.tile([P, S], F32, tag="sml", name="pswk", bufs=1)
                        for hi in range(HT):
                            nc.tensor.matmul(
                                psw[:, :S], q_hl[:, 0, hi, qsl], wsT[:, hi, :],
                                start=(hi == 0), stop=(hi == HT - 1))
                        wmax = tiny.tile([P, 1], F32, tag="c1", name="wmax")
                        nc.vector.reduce_max(wmax[:, :], psw[:, :S],
                                             axis=AXL.X)
                        nc.vector.tensor_scalar_mul(wmax[:, :], wmax[:, :],
                                                    -inv_sqrt_h)
                        ew = tiny.tile([P, S], F32, tag="c64", name="ew")
                        nc.scalar.activation(ew[:, :], psw[:, :S], AF.Exp,
                                             bias=wmax[:, :1],
                                             scale=inv_sqrt_h)
                        zw = tiny.tile([P, 1], F32, tag="c1", name="zw")
                        nc.vector.reduce_sum(zw[:, :], ew[:, :], axis=AXL.X)
                        nc.vector.reciprocal(zw[:, :], zw[:, :])
                        nc.vector.tensor_scalar(ew[:, :], ew[:, :],
                                                zw[:, :1], None, op0=ALU.mult)
                        pset = ps_tr.tile([S, P], F32, tag="tr", name="ewtp")
                        nc.tensor.transpose(out=pset[:S, :], in_=ew[:, :],
                                            identity=ident[:])
                        ewT = cst.tile([S, P], BF16, name=f"ewT{bt}")
                        nc.vector.tensor_copy(ewT[:, :], pset[:S, :])
                        ewT_pre.append(ewT)

                        psg = ps_sml.tile([P, 64], F32, tag="sml", name="psg", bufs=1)
                        for hi in range(HT):
                            nc.tensor.matmul(
                                psg[:, :64], q_hl[:, 0, hi, qsl],
                                gw1[:, hi, :],
                                start=(hi == 0), stop=(hi == HT - 1))
                        hid = tiny.tile([P, 64], F32, tag="c64", name="hid")
                        nc.vector.tensor_add(hid[:, :], psg[:, :64],
                                             b1bc[:, :])
                        nc.scalar.activation(hid[:, :], hid[:, :], AF.Silu)
                        psht = ps_tr.tile([64, P], F32, tag="tr", name="hidtp")
                        nc.tensor.transpose(out=psht[:64, :], in_=hid[:, :],
                                            identity=ident[:])
                        hidT = tiny.tile([64, P], BF16, tag="c128",
                                         name="hidT")
                        nc.vector.tensor_copy(hidT[:, :], psht[:64, :])
                        psg2 = ps_sml.tile([P, 3], F32, tag="sml", name="psg2", bufs=1)
                        nc.tensor.matmul(psg2[:, :3], hidT[:, :], gw2[:, :],
                                         start=True, stop=True)
                        gl = cst.tile([P, 3], F32, name=f"gl{bt}")
                        nc.vector.tensor_add(gl[:, :], psg2[:, :3], b2bc[:, :])
                        gmax = tiny.tile([P, 1], F32, tag="c1", name="gmax")
                        nc.vector.reduce_max(gmax[:, :], gl[:, :], axis=AXL.X)
                        nc.vector.tensor_scalar_mul(gmax[:, :], gmax[:, :],
                                                    -1.0)
                        nc.scalar.activation(gl[:, :], gl[:, :], AF.Exp,
                                             bias=gmax[:, :1])
                        gz = tiny.tile([P, 1], F32, tag="c1", name="gz")
                        nc.vector.reduce_sum(gz[:, :], gl[:, :], axis=AXL.X)
                        nc.vector.reciprocal(gz[:, :], gz[:, :])
                        nc.vector.tensor_scalar(gl[:, :], gl[:, :],
                                                gz[:, :1], None, op0=ALU.mult)
                        gw_pre.append(gl)

                    # =========================================================
                    # Phase SIM-E: own queries x all episodic keys
                    # =========================================================
                    cand_v_e = [cst.tile([P, 128], F32, name=f"cve{bt}")
                                for bt in range(NBT)]
                    cand_i_e = [cst.tile([P, 128], F32, name=f"cie{bt}")
                                for bt in range(NBT)]
                    with tc.tile_pool(name="ph_se", bufs=2) as ph_se, \
                            tc.tile_wait_until(0.55):
                        for slab in range(NCORES):
                            base = slab * (2 * H + 2)
                            for khalf in range(2):
                                csl = slice(khalf * 256, (khalf + 1) * 256)
                                ekg = ph_se.tile([P, 2, HT, 256], BF16,
                                                 tag="ekg", name="ekg")
                                nc.gpsimd.dma_start(
                                    ekg[:],
                                    ag_ek_out[base:base + 2 * H,
                                              csl].rearrange(
                                        "(s hi p) c -> p s hi c", p=P, s=2))
                                srg = rows.tile([2, 512], BF16, tag="srg",
                                                name="srg")
                                nc.gpsimd.dma_start(
                                    srg[:2, :256],
                                    ag_ek_out[base + 2 * H:base + 2 * H + 2,
                                              csl])
                                sfull = rows.tile([1, 512], F32, tag="sfl",
                                                  name="sfull")
                                nc.vector.tensor_copy(sfull[:1, :256],
                                                      srg[0:1, :256])
                                nc.vector.tensor_tensor(
                                    out=sfull[:1, :256], in0=sfull[:1, :256],
                                    in1=srg[1:2, :256], op=ALU.add)
                                bc_e = sqp.tile([P, 512], F32, tag="sq",
                                                name="bc_e")
                                nc.gpsimd.partition_broadcast(
                                    bc_e[:, :256], sfull[:1, :256])
                                cid = 2 * slab + khalf
                                for bt in range(NBT):
                                    qsl = slice(bt * P, (bt + 1) * P)
                                    ps = ps_mm.tile([P, 512], F32, tag="mm",
                                                    name="ps_se")
                                    mm3(ps[:, :256], q_hl, ekg, s_sl=qsl)
                                    sc = simcp.tile([P, 256], F32, tag="sime",
                                                    name="sc_e")
                                    nc.vector.tensor_mul(sc[:], ps[:, :256],
                                                         bc_e[:, :256])
                                    mx = tiny.tile([P, 8], F32, tag="mx",
                                                   name="mx_e")
                                    mi = tiny.tile([P, 8], U32, tag="mi",
                                                   name="mi_e")
                                    nc.vector.max(out=mx[:], in_=sc[:])
                                    nc.vector.max_index(out=mi[:],
                                                        in_max=mx[:],
                                                        in_values=sc[:])
                                    nc.vector.tensor_copy(
                                        cand_v_e[bt][:,
                                                     cid * 8:(cid + 1) * 8],
                                        mx[:])
                                    mif = tiny.tile([P, 8], F32, tag="mif",
                                                    name="mif_e")
                                    nc.vector.tensor_copy(mif[:], mi[:])
                                    nc.vector.tensor_scalar_add(
                                        cand_i_e[bt][:,
                                                     cid * 8:(cid + 1) * 8],
                                        mif[:], float(cid * 256))

                    # --- episodic top-8 merge + gather + weighted sum ---
                    acc_e_b = [ph_acc.tile([P, H], BF16, tag=f"acce{bt}",
                                           name=f"acce{bt}")
                               for bt in range(NBT)]
                    for bt in range(NBT):
                        top8 = tiny.tile([P, 8], F32, tag="c8", name="top8")
                        nc.vector.max(out=top8[:], in_=cand_v_e[bt][:])
                        idxf = tiny.tile([P, 8], F32, tag="c8b", name="idxf")
                        eqm = simcp.tile([P, 128], F32, tag="eqm", name="eqm")
                        for kk in range(EP_K):
                            nc.vector.tensor_scalar(
                                eqm[:, :], cand_v_e[bt][:],
                                top8[:, kk:kk + 1], None, op0=ALU.is_equal)
                            nc.vector.tensor_tensor(
                                out=eqm[:, :], in0=eqm[:, :],
                                in1=cand_i_e[bt][:], op=ALU.mult)
                            nc.vector.reduce_sum(idxf[:, kk:kk + 1],
                                                 eqm[:, :], axis=AXL.X)
                        idxu = tiny.tile([P, 8], U32, tag="c8u", name="idxu")
                        nc.vector.tensor_copy(idxu[:, :], idxf[:, :])
                        sc8 = tiny.tile([P, 8], F32, tag="c8c", name="sc8")
                        nc.vector.tensor_scalar(
                            sc8[:, :], top8[:, :], invq_p[:, bt:bt + 1], None,
                            op0=ALU.mult)
                        negm = tiny.tile([P, 1], F32, tag="c1", name="negm")
                        nc.vector.tensor_scalar_mul(negm[:, :], sc8[:, 0:1],
                                                    -1.0)
                        nc.scalar.activation(sc8[:, :], sc8[:, :], AF.Exp,
                                             bias=negm[:, :1])
                        zs = tiny.tile([P, 1], F32, tag="c1", name="zs")
                        nc.vector.reduce_sum(zs[:, :], sc8[:, :], axis=AXL.X)
                        nc.vector.reciprocal(zs[:, :], zs[:, :])
                        nc.vector.tensor_scalar(zs[:, :], zs[:, :],
                                                gw_pre[bt][:, 1:2], None,
                                                op0=ALU.mult)
                        nc.vector.tensor_scalar(sc8[:, :], sc8[:, :],
                                                zs[:, :1], None, op0=ALU.mult)
                        acc = simcp.tile([P, H], F32, tag="acc", name="acc_e",
                                         bufs=2)
                        nc.vector.memset(acc[:, :], 0.0)
                        for kk in range(EP_K):
                            g = gath.tile([P, H], BF16, tag="g", name="g_e")
                            nc.gpsimd.indirect_dma_start(
                                out=g[:, :], out_offset=None, in_=ep_store_b,
                                in_offset=bass.IndirectOffsetOnAxis(
                                    ap=idxu[:, kk:kk + 1], axis=0))
                            nc.vector.scalar_tensor_tensor(
                                out=acc[:, :], in0=g[:, :],
                                scalar=sc8[:, kk:kk + 1],
                                in1=acc[:, :], op0=ALU.mult, op1=ALU.add)
                        nc.vector.tensor_copy(acc_e_b[bt][:, :], acc[:, :])

                # ==== ph_qhl closed: q_hl freed ====
                # =============================================================
                # Phase SIM-S: ALL queries x local semantic keys (kc-outer)
                # =============================================================
                with tc.tile_pool(name="ph_ss", bufs=1) as ph_ss:
                    cand_sv = ph_ss.tile([P, 32 * 32], F32, tag="csv",
                                         name="cand_sv")
                    cand_si = ph_ss.tile([P, 32 * 32], F32, tag="csi",
                                         name="cand_si")
                    for kc in range(4):
                        msl = slice(kc * 512, (kc + 1) * 512)
                        ksc = ph_ss.tile([P, 2, HT, 512], BF16, tag="ksc",
                                         name="ksc")
                        nc.sync.dma_start(
                            ksc[:],
                            ks_dram[:, msl].rearrange(
                                "(s hi p) c -> p s hi c", p=P, s=2))
                        for rq2 in range(16):
                            slabq = rq2 // 2
                            base = slabq * 2 * H
                            col0 = (rq2 % 2) * 256
                            qsg = ph_ss.tile([P, 2, HT, 256], BF16, tag="qsg",
                                             name="qsg", bufs=2)
                            nc.gpsimd.dma_start(
                                qsg[:],
                                ag_qs_out[base:base + 2 * H,
                                          col0:col0 + 256].rearrange(
                                    "(s hi p) c -> p s hi c", p=P, s=2))
                            for rq in range(2):
                                rqt = rq2 * 2 + rq
                                qssl = slice(rq * P, (rq + 1) * P)
                                ps = ps_mm.tile([P, 512], F32, tag="mm",
                                                name="ps_ss")
                                mm3(ps[:], qsg, ksc, s_sl=qssl)
                                sc = simcp.tile([P, 512], F32, tag="scs",
                                                name="sc_s")
                                nc.vector.tensor_mul(sc[:, :], ps[:],
                                                     bc_ks[kc][:, :])
                                mx = tiny.tile([P, 8], F32, tag="mx",
                                               name="mx_s")
                                mi = tiny.tile([P, 8], U32, tag="mi",
                                               name="mi_s")
                                nc.vector.max(out=mx[:], in_=sc[:])
                                nc.vector.max_index(out=mi[:], in_max=mx[:],
                                                    in_values=sc[:])
                                wsl = slice(rqt * 32 + kc * 8,
                                            rqt * 32 + (kc + 1) * 8)
                                nc.vector.tensor_copy(cand_sv[:, wsl], mx[:])
                                mif = tiny.tile([P, 8], F32, tag="mif",
                                                name="mif_s")
                                nc.vector.tensor_copy(mif[:], mi[:])
                                nc.vector.tensor_scalar_add(
                                    cand_si[:, wsl], mif[:], float(kc * 512))

                    # local top-4 per query, global index, ship via a2a
                    for rqt in range(32):
                        wsl = slice(rqt * 32, (rqt + 1) * 32)
                        top8 = tiny.tile([P, 8], F32, tag="c8", name="top8l")
                        nc.vector.max(out=top8[:], in_=cand_sv[:, wsl])
                        idxf = tiny.tile([P, 8], F32, tag="c8b", name="idxfl")
                        eqm = simcp.tile([P, 32], F32, tag="eqs", name="eqml")
                        for kk in range(SEM_K):
                            nc.vector.tensor_scalar(
                                eqm[:, :], cand_sv[:, wsl],
                                top8[:, kk:kk + 1], None, op0=ALU.is_equal)
                            nc.vector.tensor_tensor(
                                out=eqm[:, :], in0=eqm[:, :],
                                in1=cand_si[:, wsl], op=ALU.mult)
                            nc.vector.reduce_sum(idxf[:, kk:kk + 1],
                                                 eqm[:, :], axis=AXL.X)
                        p4 = tiny.tile([P, 8], F32, tag="p4", name="p4")
                        nc.vector.tensor_copy(p4[:, 0:4], top8[:, 0:4])
                        nc.vector.tensor_scalar(
                            p4[:, 4:8], idxf[:, 0:4], kb_bc[:, 0:1], None,
                            op0=ALU.add)
                        nc.gpsimd.dma_start(cand_in[rqt * P:(rqt + 1) * P, :],
                                          p4[:, :])
                nc.gpsimd.collective_compute(
                    "AllToAll", ALU.bypass,
                    replica_groups=[list(range(NCORES))],
                    ins=[cand_in.opt()], outs=[cand_out.opt()])

                # =============================================================
                # Phase FINAL
                # =============================================================
                with tc.tile_pool(name="fin", bufs=1) as fin:
                    def transpose_b(src_b, dst):
                        for hi in range(HT):
                            pst = ps_tr.tile([P, P], BF16, tag="trb16",
                                             name="trp", bufs=2)
                            nc.tensor.transpose(
                                out=pst[:], in_=src_b[:, hi * P:(hi + 1) * P],
                                identity=ident_b[:])
                            nc.vector.tensor_copy(dst[:, hi, :], pst[:])

                    def val_stage(w_ap, accT_list, out_tiles, mode,
                                  gscale=None):
                        for jc in range(4):
                            jsl = slice(jc * 512, (jc + 1) * 512)
                            wvs = []
                            for h2 in range(2):
                                wv = fin.tile([P, 8, 512], BF16, tag="wv",
                                              name="wv", bufs=2)
                                nc.sync.dma_start(
                                    wv[:], w_ap[jc][:, h2 * 8:(h2 + 1) * 8, :])
                                wvs.append(wv)
                            for bt in range(NBT):
                                ps = ps_mm.tile([P, 512], F32, tag="mm",
                                                name="ps_v")
                                for hi in range(HT):
                                    nc.tensor.matmul(
                                        ps[:], accT_list[bt][:, hi, :],
                                        wvs[hi // 8][:, hi % 8, :],
                                        start=(hi == 0),
                                        stop=(hi == HT - 1))
                                if mode == "set":
                                    nc.vector.tensor_copy(
                                        out_tiles[bt][:, jsl], ps[:])
                                else:
                                    nc.vector.tensor_add(
                                        out_tiles[bt][:, jsl],
                                        out_tiles[bt][:, jsl], ps[:])

                    # e chain: tmp_e = acc_e @ W_ev
                    accT = [fin.tile([P, HT, P], BF16, tag="accT",
                                     name=f"accT{bt}", bufs=4)
                            for bt in range(NBT)]
                    for bt in range(NBT):
                        transpose_b(acc_e_b[bt], accT[bt])
                    tmp_e = [fin.tile([P, H], BF16, tag="t16",
                                      name=f"tmpe{bt}", bufs=4)
                             for bt in range(NBT)]
                    val_stage(wev_t, accT, tmp_e, "set")
                    accT2 = [fin.tile([P, HT, P], BF16, tag="accT",
                                      name=f"accT2{bt}", bufs=4)
                             for bt in range(NBT)]
                    for bt in range(NBT):
                        transpose_b(tmp_e[bt], accT2[bt])

                    # bl = gl0 * w_out
                    bl = [fin.tile([P, H], F32, tag="f32b", name=f"bl{bt}",
                                   bufs=4)
                          for bt in range(NBT)]
                    for jc in range(4):
                        wvw = fin.tile([S, 512], BF16, tag="wvw", name="wvw",
                                       bufs=2)
                        nc.sync.dma_start(wvw[:S, :],
                                          work_b[:, jc * 512:(jc + 1) * 512])
                        jsl = slice(jc * 512, (jc + 1) * 512)
                        for bt in range(NBT):
                            ps = ps_mm.tile([P, 512], F32, tag="mm",
                                            name="ps_w")
                            nc.tensor.matmul(ps[:], ewT_pre[bt][:, :],
                                             wvw[:S, :], start=True,
                                             stop=True)
                            nc.vector.tensor_scalar(
                                bl[bt][:, jsl], ps[:], gw_pre[bt][:, 0:1],
                                None, op0=ALU.mult)

                    # bl += tmp_e @ W_eo
                    val_stage(weo_t, accT2, bl, "add")

                    # --- semantic merge + gather (after AllToAll) ---
                    acc_s_b = [ph_acc.tile([P, H], BF16, tag=f"accs{bt}",
                                           name=f"accs{bt}")
                               for bt in range(NBT)]
                    for bt in range(NBT):
                        c32v = simcp.tile([P, 32], F32, tag="eqs",
                                          name="c32v")
                        c32i = simcp.tile([P, 32], F32, tag="eqs2",
                                          name="c32i")
                        for r in range(NCORES):
                            c8 = tiny.tile([P, 8], F32, tag="p4", name="c8in")
                            nc.gpsimd.dma_start(
                                c8[:, :],
                                cand_out[r * BL + bt * P:
                                         r * BL + (bt + 1) * P, :])
                            nc.vector.tensor_copy(c32v[:, r * 4:(r + 1) * 4],
                                                  c8[:, 0:4])
                            nc.vector.tensor_copy(c32i[:, r * 4:(r + 1) * 4],
                                                  c8[:, 4:8])
                        top8 = tiny.tile([P, 8], F32, tag="c8", name="top8s")
                        nc.vector.max(out=top8[:], in_=c32v[:])
                        idxf = tiny.tile([P, 8], F32, tag="c8b", name="idxfs")
                        eqs = simcp.tile([P, 32], F32, tag="eqs3", name="eqs")
                        for kk in range(SEM_K):
                            nc.vector.tensor_scalar(
                                eqs[:, :], c32v[:, :], top8[:, kk:kk + 1],
                                None, op0=ALU.is_equal)
                            nc.vector.tensor_tensor(out=eqs[:, :],
                                                    in0=eqs[:, :],
                                                    in1=c32i[:, :],
                                                    op=ALU.mult)
                            nc.vector.reduce_sum(idxf[:, kk:kk + 1],
                                                 eqs[:, :], axis=AXL.X)
                        idxu = tiny.tile([P, 8], U32, tag="c8u", name="idxus")
                        nc.vector.tensor_copy(idxu[:, 0:4], idxf[:, 0:4])
                        sc4 = tiny.tile([P, 4], F32, tag="c4", name="sc4")
                        nc.vector.tensor_scalar(
                            sc4[:, :], top8[:, 0:4], invqs_p[:, bt:bt + 1],
                            None, op0=ALU.mult)
                        negm = tiny.tile([P, 1], F32, tag="c1", name="negms")
                        nc.vector.tensor_scalar_mul(negm[:, :], sc4[:, 0:1],
                                                    -1.0)
                        nc.scalar.activation(sc4[:, :], sc4[:, :], AF.Exp,
                                             bias=negm[:, :1])
                        zs = tiny.tile([P, 1], F32, tag="c1", name="zss")
                        nc.vector.reduce_sum(zs[:, :], sc4[:, :], axis=AXL.X)
                        nc.vector.reciprocal(zs[:, :], zs[:, :])
                        nc.vector.tensor_scalar(zs[:, :], zs[:, :],
                                                gw_pre[bt][:, 2:3], None,
                                                op0=ALU.mult)
                        nc.vector.tensor_scalar(sc4[:, :], sc4[:, :],
                                                zs[:, :1], None, op0=ALU.mult)
                        acc = simcp.tile([P, H], F32, tag="acc", name="acc_s",
                                         bufs=2)
                        nc.vector.memset(acc[:, :], 0.0)
                        for kk in range(SEM_K):
                            g = gath.tile([P, H], BF16, tag="g", name="g_s")
                            nc.gpsimd.indirect_dma_start(
                                out=g[:, :], out_offset=None,
                                in_=sem_values_b,
                                in_offset=bass.IndirectOffsetOnAxis(
                                    ap=idxu[:, kk:kk + 1], axis=0))
                            nc.vector.scalar_tensor_tensor(
                                out=acc[:, :], in0=g[:, :],
                                scalar=sc4[:, kk:kk + 1],
                                in1=acc[:, :], op0=ALU.mult, op1=ALU.add)
                        nc.vector.tensor_copy(acc_s_b[bt][:, :], acc[:, :])

                    # bl += acc_s @ W_so
                    accT_s = [fin.tile([P, HT, P], BF16, tag="accT",
                                       name=f"accTs{bt}", bufs=4)
                              for bt in range(NBT)]
                    for bt in range(NBT):
                        transpose_b(acc_s_b[bt], accT_s[bt])
                    val_stage(wso_t, accT_s, bl, "add")

                    # xo = bl @ W_ro; out = LN(xo)*gamma+beta
                    blb = [fin.tile([P, H], BF16, tag="t16", name=f"blb{bt}",
                                    bufs=4)
                           for bt in range(NBT)]
                    for bt in range(NBT):
                        nc.vector.tensor_copy(blb[bt][:, :], bl[bt][:, :])
                    accT_bl = [fin.tile([P, HT, P], BF16, tag="accT",
                                        name=f"accTb{bt}", bufs=4)
                               for bt in range(NBT)]
                    for bt in range(NBT):
                        transpose_b(blb[bt], accT_bl[bt])
                    xo = [fin.tile([P, H], F32, tag="f32b", name=f"xo{bt}",
                                   bufs=4)
                          for bt in range(NBT)]
                    val_stage(wro_t, accT_bl, xo, "set")

                    for bt in range(NBT):
                        x = xo[bt]
                        mu = tiny.tile([P, 1], F32, tag="c1", name="mu")
                        nc.vector.reduce_sum(mu[:, :], x[:, :], axis=AXL.X)
                        nc.vector.tensor_scalar_mul(mu[:, :], mu[:, :],
                                                    -1.0 / H)
                        nc.vector.tensor_scalar(x[:, :], x[:, :], mu[:, :1],
                                                None, op0=ALU.add)
                        sqx = simcp.tile([P, H], F32, tag="acc", name="sqx",
                                         bufs=2)
                        vs = tiny.tile([P, 1], F32, tag="c1", name="vs")
                        nc.scalar.activation(sqx[:, :], x[:, :], AF.Square,
                                             accum_out=vs[:, :1])
                        nc.vector.tensor_scalar_mul(vs[:, :], vs[:, :],
                                                    1.0 / H)
                        nc.vector.tensor_scalar_add(vs[:, :], vs[:, :],
                                                    LN_EPS)
                        nc.scalar.sqrt(vs[:, :], vs[:, :])
                        nc.vector.reciprocal(vs[:, :], vs[:, :])
                        nc.vector.tensor_scalar(x[:, :], x[:, :], vs[:, :1],
                                                None, op0=ALU.mult)
                        for jc in range(4):
                            jsl = slice(jc * 512, (jc + 1) * 512)
                            gbch = sqp.tile([P, 512], F32, tag="sq",
                                            name="gbch")
                            grow = rows.tile([1, 512], F32, tag="crow",
                                             name="grow")
                            nc.sync.dma_start(grow[:1, :],
                                              ln_gamma[None, jsl])
                            nc.gpsimd.partition_broadcast(gbch[:, :],
                                                          grow[:1, :])
                            nc.vector.tensor_mul(x[:, jsl], x[:, jsl],
                                                 gbch[:, :])
                            bbch = sqp.tile([P, 512], F32, tag="sq",
                                            name="bbch")
                            brow = rows.tile([1, 512], F32, tag="crow",
                                             name="brow")
                            nc.sync.dma_start(brow[:1, :],
                                              ln_beta[None, jsl])
                            nc.gpsimd.partition_broadcast(bbch[:, :],
                                                          brow[:1, :])
                            nc.vector.tensor_add(x[:, jsl], x[:, jsl],
                                                 bbch[:, :])
                        nc.gpsimd.dma_start(out_s[bt * P:(bt + 1) * P, :],
                                          x[:, :])

    nc.finalize()
    return nc


_NC_CACHE = None


def _bf16_split(x):
    h = x.astype(ml_dtypes.bfloat16)
    l = (x - h.astype(np.float32)).astype(ml_dtypes.bfloat16)
    return h, l


def _tile_sel_weight(w):
    """[H, H] f32 -> [j, p, 2, hi, 128] bf16 hi/lo tiled."""
    h, l = _bf16_split(w)
    out = np.empty((HT, P, 2, HT, P), dtype=ml_dtypes.bfloat16)
    hr = h.reshape(HT, P, HT, P)   # [hi, p, j, c]
    lr = l.reshape(HT, P, HT, P)
    out[:, :, 0] = hr.transpose(2, 1, 0, 3)
    out[:, :, 1] = lr.transpose(2, 1, 0, 3)
    return np.ascontiguousarray(out)


def _tile_val_weight(w):
    """[H, H] f32 -> [jc, p, hi, 512] bf16."""
    b = w.astype(ml_dtypes.bfloat16)
    r = b.reshape(HT, P, 4, 512)   # [hi, p, jc, c]
    return np.ascontiguousarray(r.transpose(2, 1, 0, 3))


def _split_T(x):
    """[R, H] f32 -> [p, 2, hi, R] bf16 (transposed hi/lo)."""
    h, l = _bf16_split(x)
    R = x.shape[0]
    out = np.empty((P, 2, HT, R), dtype=ml_dtypes.bfloat16)
    out[:, 0] = h.T.reshape(HT, P, R).transpose(1, 0, 2)
    out[:, 1] = l.T.reshape(HT, P, R).transpose(1, 0, 2)
    return np.ascontiguousarray(out)


def kernel(**inputs) -> np.ndarray:
    global _NC_CACHE
    if _NC_CACHE is None:
        _NC_CACHE = build()
    nc = _NC_CACHE

    f32 = lambda x: np.ascontiguousarray(np.asarray(x), dtype=np.float32)
    query = f32(inputs["query"])
    ep_store = f32(inputs["ep_store"])
    sem_keys = f32(inputs["sem_keys"])
    work_slots = f32(inputs["work_slots"])

    shared = {
        "wq_t": _tile_sel_weight(f32(inputs["W_query"])),
        "wek_t": _tile_sel_weight(f32(inputs["W_ek"])),
        "wsq_t": _tile_sel_weight(f32(inputs["W_sq"])),
        "wsk_t": _tile_sel_weight(f32(inputs["W_sk"])),
        "wev_t": _tile_val_weight(f32(inputs["W_ev"])),
        "weo_t": _tile_val_weight(f32(inputs["W_eo"])),
        "wso_t": _tile_val_weight(f32(inputs["W_so"])),
        "wro_t": _tile_val_weight(f32(inputs["W_ro"])),
        "ep_store_b": ep_store.astype(ml_dtypes.bfloat16),
        "sem_values_b": f32(inputs["sem_values"]).astype(ml_dtypes.bfloat16),
        "wsT_b": np.ascontiguousarray(
            work_slots.T.astype(ml_dtypes.bfloat16).reshape(HT, P, S)
            .transpose(1, 0, 2)),
        "work_b": work_slots.astype(ml_dtypes.bfloat16),
        "gw1_b": np.ascontiguousarray(
            f32(inputs["gate_W1"]).astype(ml_dtypes.bfloat16)
            .reshape(HT, P, 64).transpose(1, 0, 2)),
        "gw2_b": f32(inputs["gate_W2"]).astype(ml_dtypes.bfloat16),
        "ep_imp": f32(inputs["ep_importance"]),
        "ep_ts": f32(inputs["ep_timestamps"]),
        "gate_b1": f32(inputs["gate_b1"]),
        "gate_b2": f32(inputs["gate_b2"]),
        "ln_gamma": f32(inputs["ln_gamma"]),
        "ln_beta": f32(inputs["ln_beta"]),
    }

    in_maps = []
    for c in range(NCORES):
        m = dict(shared)
        m["qtin"] = _split_T(query[c * BL:(c + 1) * BL])
        m["eptin"] = _split_T(ep_store[c * NL:(c + 1) * NL])
        m["sktin"] = _split_T(sem_keys[c * ML:(c + 1) * ML])
        m["ep_imp_s"] = f32(inputs["ep_importance"][c * NL:(c + 1) * NL])
        m["ep_ts_s"] = f32(inputs["ep_timestamps"][c * NL:(c + 1) * NL])
        m["key_base"] = np.array([c * ML], dtype=np.float32)
        in_maps.append(m)

    res = run_bass_kernel_spmd(nc, in_maps, core_ids=list(range(NCORES)))
    return np.concatenate([res.results[c]["out_s"] for c in range(NCORES)],
                          axis=0)
